# revision 32
# baseline (speedup 1.0000x reference)
"""HGT link predictor on 8 trn2 NeuronCores.

Sharding: nodes split 8 ways per type (2500/core, padded to 2560).
Params replicated. Edges partitioned by destination core, sorted by dst,
packed into 128-edge chunks within 128-dst-node windows.

v2 design:
- All matmul operands bf16 (PSUM accumulates fp32): 4x PE throughput.
- a_rel folded into per-relation q projections (dst side, local);
  m_rel applied post-aggregation as block-diagonal matmuls (dst side,
  local). The halo exchange then only carries RAW k/v per source type:
  two bf16 AllGathers per layer ([2560,512] -> [20480,512] each), 8x
  less wire than the baseline's per-relation folded fp32 tables.
- Scatter-add via one-hot matmuls producing FEATURE-major aggregates
  (msg as stationary operand), so m_rel and Wa matmuls consume them
  directly with no per-window transposes; softmax denominator rides as
  8 extra stationary columns.
- Collective outputs use addr_space="Shared".
Per layer: kv projections -> AllGather(t0), AllGather(t1) overlapped
with q' projections -> per-relation edge phase (gather kv/q' rows,
logits via mul+segmented reduce, exp, one-hot scatter matmuls,
normalize, m_rel, gelu, Wa, gated skip + residual + LayerNorm + relu).
"""
import math
import numpy as np

import concourse.bacc as bacc
import concourse.bass as bass
import concourse.mybir as mybir
import concourse.tile as tile
from concourse.bass_utils import run_bass_kernel_spmd
from concourse.library_config import mlp

F32 = mybir.dt.float32
BF16 = mybir.dt.bfloat16
I16 = mybir.dt.int16
AF = mybir.ActivationFunctionType
OP = mybir.AluOpType

T, R, L = 3, 4, 2
H, HEADS, D, FIN, OUT = 256, 8, 32, 128, 128
SRC_T = (0, 1, 1, 1)
DST_T = (1, 0, 1, 2)
LN_EPS = 1e-5
NC = 8
N = 20000
DBG = False
PROJ_ORDER = (0, 1, 2)
EDGE_ORDER = (0, 1, 3, 2)
NL = N // NC          # 2500 real local nodes per type
NT = 20               # node tiles of 128
NLP = NT * 128        # 2560 padded local nodes
NWIN = NT             # dst windows of 128 local nodes
GWIN = 2              # windows per gather group
KV_W = 2 * H          # 512: [k || v] columns of a kv-table row


def _block_diag(a):
    """a: [HEADS, D, D] -> [H, H] block diagonal."""
    out = np.zeros((H, H), np.float32)
    for h in range(HEADS):
        out[h * D:(h + 1) * D, h * D:(h + 1) * D] = a[h]
    return out


def _wrap_idx(idx):
    """idx [M] -> [128, M//16] int16 wrapped in 16 partitions, replicated."""
    m = idx.shape[0]
    assert m % 16 == 0
    w = np.zeros((128, m // 16), np.int16)
    w[:16] = idx.astype(np.int16).reshape(m // 16, 16).T
    for rep in range(1, 8):
        w[16 * rep:16 * rep + 16] = w[:16]
    return w


def _preprocess(inputs):
    x = np.asarray(inputs["x"], np.float32)
    edge_index = np.asarray(inputs["edge_index"])
    Win = np.asarray(inputs["Win"], np.float32)
    b_in = np.asarray(inputs["b_in"], np.float32)
    Wk = np.asarray(inputs["Wk"], np.float32); bk = np.asarray(inputs["bk"], np.float32)
    Wq = np.asarray(inputs["Wq"], np.float32); bq = np.asarray(inputs["bq"], np.float32)
    Wv = np.asarray(inputs["Wv"], np.float32); bv = np.asarray(inputs["bv"], np.float32)
    Wa = np.asarray(inputs["Wa"], np.float32); ba = np.asarray(inputs["ba"], np.float32)
    skip = np.asarray(inputs["skip"], np.float32)
    a_rel = np.asarray(inputs["a_rel"], np.float32)
    m_rel = np.asarray(inputs["m_rel"], np.float32)
    p_rel = np.asarray(inputs["p_rel"], np.float32)
    ln_g = np.asarray(inputs["ln_g"], np.float32)
    ln_b = np.asarray(inputs["ln_b"], np.float32)
    Wout = np.asarray(inputs["Wout"], np.float32)
    bout = np.asarray(inputs["bout"], np.float32)

    meta = {}
    inv_sqrt_d = 1.0 / math.sqrt(D)
    # fold a_rel (scaled) into dst-side q projections per relation
    wq_eff = np.zeros((L, R, H, H), np.float32)
    bq_eff = np.zeros((L, R, H), np.float32)
    # block-diag m_rel chunks for post-aggregation transform (lhsT layout)
    m_blk = np.zeros((L, R, 2, 128, 128), np.float32)
    for l in range(L):
        for r in range(R):
            dt = DST_T[r]
            at = _block_diag(np.transpose(a_rel[l, r], (0, 2, 1))
                             * (p_rel[l, r] * inv_sqrt_d)[:, None, None])
            wq_eff[l, r] = Wq[l, dt] @ at
            bq_eff[l, r] = bq[l, dt] @ at
            mb = _block_diag(m_rel[l, r])
            m_blk[l, r, 0] = mb[0:128, 0:128]
            m_blk[l, r, 1] = mb[128:256, 128:256]
    beta = 1.0 / (1.0 + np.exp(-skip))          # [L, T]
    g = beta / (2.0 - beta)
    wa_eff = Wa * g[:, :, None, None]
    ba_eff = ba * g[:, :, None]
    meta["eps_eff"] = (LN_EPS / (2.0 - beta) ** 2).tolist()

    meta["use_bias"] = dict(
        bin_=bool(np.any(b_in)), bq=bool(np.any(bq_eff)),
        bkv=bool(np.any(bk[:, :2])) or bool(np.any(bv[:, :2])),
        ba=bool(np.any(ba_eff)), bout=bool(np.any(bout)),
        lng=not np.allclose(ln_g, 1.0), lnb=bool(np.any(ln_b)),
    )

    def bcast(v):
        # [..., F] -> [..., 128, F]: per-feature vectors replicated across partitions
        return np.ascontiguousarray(
            np.broadcast_to(v[..., None, :], v.shape[:-1] + (128, v.shape[-1])))

    # edge partitioning ---------------------------------------------------
    win_edges = [[] for _ in range(NC)]   # [c][r][w] -> (src_rows, dst_loc)
    kch_need = 1
    for c in range(NC):
        rel = []
        for r in range(R):
            src = edge_index[r, 0].astype(np.int64)
            dst = edge_index[r, 1].astype(np.int64)
            sel = (dst // NL) == c
            s, d = src[sel], dst[sel] - c * NL
            o = np.argsort(d, kind="stable")
            s, d = s[o], d[o]
            wins = []
            for w in range(NWIN):
                m = (d // 128) == w
                sw, dw = s[m], d[m]
                kch_need = max(kch_need, (len(sw) + 127) // 128)
                wins.append((sw, dw))
            rel.append(wins)
        win_edges[c] = rel
    KCH = kch_need
    meta["KCH"] = KCH
    NCHUNK = NWIN * KCH
    NIDX_R = NCHUNK * 128

    per_core = []
    for c in range(NC):
        oh = np.zeros((R, NCHUNK, 128, 128), np.float32)
        kv_idx = np.zeros((R, NIDX_R), np.int64)
        qi_idx = np.zeros((R, NIDX_R), np.int64)
        for r in range(R):
            for w in range(NWIN):
                sw, dw = win_edges[c][r][w]
                ne = len(sw)
                base = w * KCH * 128
                # src node n (global) -> kv-table row (n//NL)*NLP + n%NL
                kv_idx[r, base:base + ne] = (sw // NL) * NLP + (sw % NL)
                qi_idx[r, base:base + ne] = dw
                ch = base // 128 + np.arange(ne) // 128
                oh[r, ch, np.arange(ne) % 128, dw - w * 128] = 1.0
        # partition-major one-hot: [R, 128(edge), NCHUNK, 128(col)]
        oh_pm = np.ascontiguousarray(oh.transpose(0, 2, 1, 3))
        xc = np.zeros((T, 128, NLP), np.float32)
        xc[:, :, :NL] = x[:, c * NL:(c + 1) * NL, :].transpose(0, 2, 1)
        per_core.append(dict(
            xT_h=_bf(xc),
            oh=_bf(oh_pm),
            kv_idx=np.stack([_wrap_idx(kv_idx[r]) for r in range(R)]),
            qi_idx=np.stack([_wrap_idx(qi_idx[r]) for r in range(R)]),
        ))

    shared = dict(
        win=_bf(Win),                                     # [3,128,256]
        wk=_bf(Wk[:, :2]), wv=_bf(Wv[:, :2]),             # [L,2,256,256]
        wq=_bf(wq_eff), wa=_bf(wa_eff),
        m_blk=_bf(m_blk),
        wout=_bf(Wout),
        ident=np.eye(128, dtype=np.float32),
        identb=_bf(np.eye(128, dtype=np.float32)),
        bin_b=bcast(b_in), bq_b=bcast(bq_eff),
        bk_b=bcast(bk[:, :2]), bv_b=bcast(bv[:, :2]),
        ba_b=bcast(ba_eff), bout_b=bcast(bout),
        lng_b=bcast(ln_g), lnb_b=bcast(ln_b),
    )
    return shared, per_core, meta


def _bf(a):
    import ml_dtypes
    return np.ascontiguousarray(a).astype(ml_dtypes.bfloat16)


def NIDX_R16(KCH):
    return NWIN * KCH * 128 // 16


def _build(nc, meta, shapes):
    KCH = meta["KCH"]
    NCHUNK = NWIN * KCH
    GC = GWIN * KCH                      # chunks per gather group
    NGRP = NWIN // GWIN
    ub = meta["use_bias"]
    eps_eff = meta["eps_eff"]

    def din(name, dt_):
        return nc.dram_tensor(name, shapes[name], dt_, kind="ExternalInput").ap()

    xT_h = din("xT_h", BF16); oh_d = din("oh", BF16)
    kv_idx_d = din("kv_idx", I16); qi_idx_d = din("qi_idx", I16)
    win_d = din("win", BF16)
    wk_d = din("wk", BF16); wv_d = din("wv", BF16)
    wq_d = din("wq", BF16); wa_d = din("wa", BF16)
    m_blk_d = din("m_blk", BF16)
    wout_d = din("wout", BF16)
    ident_d = din("ident", F32); identb_d = din("identb", BF16)
    bias_d = {k: din(k, F32) for k in
              ("bin_b", "bq_b", "bk_b", "bv_b", "ba_b", "bout_b", "lng_b", "lnb_b")}
    y_d = nc.dram_tensor("y", [T, NLP, OUT], F32, kind="ExternalOutput").ap()
    if DBG:
        GC_ = GWIN * meta["KCH"]
        dbg_kvg = nc.dram_tensor("dbg_kvg", [128, GC_, KV_W], BF16,
                                 kind="ExternalOutput").ap()
        dbg_qig = nc.dram_tensor("dbg_qig", [128, GC_, H], BF16,
                                 kind="ExternalOutput").ap()
        dbg_msg = nc.dram_tensor("dbg_msg", [128, GC_, H + HEADS], BF16,
                                 kind="ExternalOutput").ap()
        dbg_agg = nc.dram_tensor("dbg_agg", [128, 2, NWIN, 128], BF16,
                                 kind="ExternalOutput").ap()
        dbg_an = nc.dram_tensor("dbg_an", [128, H], BF16,
                                kind="ExternalOutput").ap()
        dbg_h = nc.dram_tensor("dbg_h", [128, T, NT, H], F32,
                               kind="ExternalOutput").ap()

    def bc32(ap2d):
        """[..., k] AP -> [..., k, 32] stride-0 broadcast AP."""
        return bass.AP(tensor=ap2d.tensor, offset=ap2d.offset,
                       ap=list(ap2d.ap) + [[0, D]])

    with tile.TileContext(nc) as tc:
        with (
            tc.tile_pool(name="persist", bufs=1) as pp,
            tc.tile_pool(name="wpool", bufs=8) as wp,
            tc.tile_pool(name="wsmall", bufs=3) as ws,
            tc.tile_pool(name="stage", bufs=2) as stg,
            tc.tile_pool(name="edge", bufs=2) as ep,
            tc.tile_pool(name="small", bufs=4) as sp,
            tc.tile_pool(name="psSC", bufs=2, space="PSUM") as psSC,
            tc.tile_pool(name="psAG", bufs=3, space="PSUM") as psAG,
            tc.tile_pool(name="psPO", bufs=3, space="PSUM") as psPO,
            tc.tile_pool(name="dram", bufs=1, space="DRAM") as dp,
        ):
            nc.gpsimd.load_library(mlp)

            ident = pp.tile([128, 128], F32, tag="ident")
            nc.sync.dma_start(ident[:], ident_d)
            identb = pp.tile([128, 128], BF16, tag="identb")
            nc.sync.dma_start(identb[:], identb_d)
            h = pp.tile([128, T, NT, H], F32, tag="h")
            agg1T = pp.tile([128, 2, NT, 128], BF16, tag="agg1T")

            kv_loc = [[dp.tile([NLP, KV_W], BF16, name=f"kv_loc{l}{t}")
                       for t in range(2)] for l in range(L)]
            kv_full = [[dp.tile([NC * NLP, KV_W], BF16, addr_space="Shared",
                                name=f"kv_full{l}{t}")
                        for t in range(2)] for l in range(L)]
            q_dram = [dp.tile([R, NLP, H], BF16, name=f"q_dram{l}")
                      for l in range(L)]

            def load_w(src_ap):
                """[256, M] bf16 dram -> [128, 2, M] sbuf tile."""
                m = src_ap.shape[-1]
                t_ = wp.tile([128, 2, m], BF16, tag="w")
                nc.sync.dma_start(t_[:], src_ap.rearrange("(kt kp) m -> kp kt m", kp=128))
                return t_

            def load_bias(src_ap):
                t_ = wp.tile([128, H], F32, tag="bias")
                nc.sync.dma_start(t_[:], src_ap)
                return t_

            # ---- input projection: h[t] = relu(xT^T @ Win + b) ----
            with nc.named_scope("inproj"):
                for t in range(T):
                    w_in = ws.tile([128, H], BF16, tag="win")
                    nc.sync.dma_start(w_in[:], win_d[t])
                    bt = load_bias(bias_d["bin_b"][t]) if ub["bin_"] else None
                    for nt in range(NT):
                        xt = ws.tile([128, 128], BF16, tag="xt")
                        nc.sync.dma_start(xt[:], xT_h[t, :, nt * 128:(nt + 1) * 128])
                        ps = psPO.tile([128, H], F32, tag="po")
                        nc.tensor.matmul(ps[:], xt[:], w_in[:], start=True, stop=True)
                        if bt is not None:
                            nc.vector.tensor_add(ps[:], ps[:], bt[:])
                        nc.scalar.activation(h[:, t, nt, :], ps[:], AF.Relu)

            def transpose_tile(src2, nt_label):
                """h tile [128, 256] f32 -> hT [128, 2, 128] bf16 (feature-major)."""
                hTt = ws.tile([128, 2, 128], BF16, tag="hTt")
                for ft in range(2):
                    tp = psSC.tile([128, 128], F32, tag="sc")
                    nc.tensor.transpose(tp[:], src2[:, ft * 128:(ft + 1) * 128], ident[:])
                    if (nt_label + ft) % 2:
                        nc.vector.tensor_copy(hTt[:, ft, :], tp[:])
                    else:
                        nc.scalar.copy(hTt[:, ft, :], tp[:])
                return hTt

            for l in range(L):
                # ---- per-type transposes + projections ----
                # type order 1,0,2: h[1] finishes first in the previous
                # layer's edge phase (r order 0,2,1,3), so its kv AllGather
                # launches earliest and overlaps the remaining edge work.
                with nc.named_scope(f"l{l}_proj"):
                    for t in PROJ_ORDER:
                        projs = []   # (wtile, btile, kind, dest-info)
                        if t < 2:
                            wkt = load_w(wk_d[l, t]); wvt = load_w(wv_d[l, t])
                            bkt = load_bias(bias_d["bk_b"][l, t]) if ub["bkv"] else None
                            bvt = load_bias(bias_d["bv_b"][l, t]) if ub["bkv"] else None
                            projs.append((wkt, bkt, "kv", 0))
                            projs.append((wvt, bvt, "kv", H))
                        for r in range(R):
                            if DST_T[r] == t:
                                wqr = load_w(wq_d[l, r])
                                bqr = load_bias(bias_d["bq_b"][l, r]) if ub["bq"] else None
                                projs.append((wqr, bqr, "q", r))
                        # staging: kv rows [128, NT/2, 512]; q rows [128, NT/2, 256]
                        for half in range(2):
                            kvstg = stg.tile([128, NT // 2, KV_W], BF16, tag="kvstg",
                                             name="kvstg") if t < 2 else None
                            qstg = {r: stg.tile([128, NT // 2, H], BF16, tag="qstg",
                                                name=f"qstg{r}")
                                    for (_, _, kind, r) in projs if kind == "q"}
                            for nti in range(NT // 2):
                                nt = half * (NT // 2) + nti
                                hTt = transpose_tile(h[:, t, nt, :], nt)
                                for (wt, bt, kind, info) in projs:
                                    ps = psPO.tile([128, H], F32, tag="po")
                                    for kt in range(2):
                                        nc.tensor.matmul(ps[:], hTt[:, kt, :],
                                                         wt[:, kt, :],
                                                         start=(kt == 0), stop=(kt == 1))
                                    dst_ = kvstg[:, nti, info:info + H] if kind == "kv" \
                                        else qstg[info][:, nti, :]
                                    if bt is not None:
                                        nc.vector.tensor_add(dst_, ps[:], bt[:])
                                    else:
                                        nc.vector.tensor_copy(dst_, ps[:])
                            r0_ = half * (NT // 2) * 128
                            r1_ = r0_ + (NT // 2) * 128
                            if t < 2:
                                nc.sync.dma_start(
                                    kv_loc[l][t][r0_:r1_, :].rearrange(
                                        "(nt kp) m -> kp nt m", kp=128),
                                    kvstg[:])
                            for r, qs in qstg.items():
                                nc.sync.dma_start(
                                    q_dram[l][r, r0_:r1_, :].rearrange(
                                        "(nt kp) m -> kp nt m", kp=128),
                                    qs[:])
                        if t < 2:
                            with nc.named_scope(f"l{l}_ag{t}"):
                                nc.gpsimd.collective_compute(
                                    "AllGather", OP.bypass,
                                    replica_groups=[list(range(NC))],
                                    ins=[kv_loc[l][t][:].opt()],
                                    outs=[kv_full[l][t][:].opt()],
                                )

                # ---- edge phase; r order: 0 (t1 agg), 1 (t0), 3 (t2), 2 (t1) ----
                s1 = sp.tile([128, NT], F32, tag="s1")
                s2 = sp.tile([128, NT], F32, tag="s2")

                def finish_type(t, l):
                    mu = sp.tile([128, NT], F32, tag="mu")
                    inv = sp.tile([128, NT], F32, tag="inv")
                    nmi = sp.tile([128, NT], F32, tag="nmi")
                    nc.vector.tensor_scalar_mul(mu[:], s1[:], 1.0 / H)
                    nc.vector.tensor_scalar_mul(inv[:], s2[:], 1.0 / H)  # mean sq
                    musq = sp.tile([128, NT], F32, tag="musq")
                    nc.vector.tensor_mul(musq[:], mu[:], mu[:])
                    nc.vector.scalar_tensor_tensor(
                        inv[:], inv[:], float(eps_eff[l][t]), musq[:],
                        OP.add, OP.subtract)              # var + eps
                    nc.scalar.activation(inv[:], inv[:], AF.Sqrt)
                    nc.vector.reciprocal(inv[:], inv[:])
                    nc.vector.scalar_tensor_tensor(
                        nmi[:], mu[:], -1.0, inv[:], OP.mult, OP.mult)
                    if ub["lng"] or ub["lnb"]:
                        lng_t = load_bias(bias_d["lng_b"][l, t])
                        lnb_t = load_bias(bias_d["lnb_b"][l, t])
                        for w in range(NT):
                            nc.scalar.activation(
                                h[:, t, w, :], h[:, t, w, :], AF.Identity,
                                bias=nmi[:, w:w + 1], scale=inv[:, w:w + 1])
                            nc.vector.tensor_mul(h[:, t, w, :], h[:, t, w, :], lng_t[:])
                            nc.vector.tensor_add(h[:, t, w, :], h[:, t, w, :], lnb_t[:])
                            nc.scalar.activation(h[:, t, w, :], h[:, t, w, :], AF.Relu)
                    else:
                        for w in range(NT):
                            nc.scalar.activation(
                                h[:, t, w, :], h[:, t, w, :], AF.Relu,
                                bias=nmi[:, w:w + 1], scale=inv[:, w:w + 1])

                for r in EDGE_ORDER:
                    _sid, _ = nc.enter_named_scope(f"l{l}_r{r}", False)
                    dt = DST_T[r]
                    st = SRC_T[r]
                    wa_t = ba_t = None
                    if r != 0:
                        wa_t = load_w(wa_d[l, dt])
                        ba_t = load_bias(bias_d["ba_b"][l, dt]) if ub["ba"] else None
                    mblk_t = ws.tile([128, 2, 128], BF16, tag="mblk")
                    nc.sync.dma_start(mblk_t[:], m_blk_d[l, r].rearrange("kt p m -> p kt m"))
                    kvi = sp.tile([128, NIDX_R16(KCH)], I16, tag="kvi")
                    qii = sp.tile([128, NIDX_R16(KCH)], I16, tag="qii")
                    nc.gpsimd.dma_start(kvi[:], kv_idx_d[r])
                    nc.gpsimd.dma_start(qii[:], qi_idx_d[r])
                    for gidx in range(NGRP):
                        ni = GC * 128
                        kvg = ep.tile([128, GC, KV_W], BF16, tag="kvg")
                        qig = ep.tile([128, GC, H], BF16, tag="qig")
                        nc.gpsimd.dma_gather(
                            kvg[:], kv_full[l][st][:],
                            kvi[:, gidx * (ni // 16):(gidx + 1) * (ni // 16)],
                            ni, ni, KV_W)
                        nc.gpsimd.dma_gather(
                            qig[:], q_dram[l][r],
                            qii[:, gidx * (ni // 16):(gidx + 1) * (ni // 16)],
                            ni, ni, H)
                        ohg = ep.tile([128, GC, 128], BF16, tag="ohg")
                        nc.sync.dma_start(ohg[:], oh_d[r, :, gidx * GC:(gidx + 1) * GC, :])
                        msg = ep.tile([128, GC, H + HEADS], BF16, tag="msg")
                        prod = ep.tile([128, GC, H], BF16, tag="prod")
                        if DBG and l == 0 and r == 0 and gidx == 0:
                            nc.sync.dma_start(dbg_kvg, kvg[:])
                            nc.sync.dma_start(dbg_qig, qig[:])
                        lg = sp.tile([128, GC, HEADS], F32, tag="lg")
                        nc.vector.tensor_mul(prod[:], qig[:], kvg[:, :, 0:H])
                        nc.vector.tensor_reduce(
                            lg[:], prod[:].rearrange("p g (hh dd) -> p g hh dd", dd=D),
                            mybir.AxisListType.X, OP.add)
                        nc.scalar.activation(msg[:, :, H:H + HEADS], lg[:], AF.Exp)
                        nc.vector.tensor_mul(
                            msg[:, :, 0:H].rearrange("p g (hh dd) -> p g hh dd", dd=D),
                            kvg[:, :, H:2 * H].rearrange("p g (hh dd) -> p g hh dd", dd=D),
                            bc32(msg[:, :, H:H + HEADS]))
                        if DBG and l == 0 and r == 0 and gidx == 0:
                            nc.sync.dma_start(dbg_msg, msg[:])
                        for wi in range(GWIN):
                            w = gidx * GWIN + wi
                            # scatter: node-major [dst, 256 agg | 8 denom]
                            pw = psSC.tile([128, 264], F32, tag="sc")
                            for kc in range(KCH):
                                ch = wi * KCH + kc
                                nc.tensor.matmul(pw[:], ohg[:, ch, :], msg[:, ch, :],
                                                 start=(kc == 0), stop=(kc == KCH - 1))
                            # rec = 1/denom  [128 dst, 8] bf16
                            # +1e-30: degree-0 dst nodes have sum 0; keep 0*recip = 0
                            recf = sp.tile([128, HEADS], F32, tag="recf")
                            nc.vector.tensor_scalar_add(recf[:], pw[:, H:H + HEADS], 1e-30)
                            rec = sp.tile([128, HEADS], BF16, tag="rec")
                            with nc.allow_low_precision(reason="softmax recip to bf16"):
                                nc.vector.reciprocal(rec[:], recf[:])
                            # normalized node-major agg, bf16
                            an = sp.tile([128, H], BF16, tag="an")
                            nc.vector.tensor_mul(
                                an[:].rearrange("p (hh dd) -> p hh dd", dd=D),
                                pw[:, 0:H].rearrange("p (hh dd) -> p hh dd", dd=D),
                                bc32(rec[:]))
                            # transpose to feature-major for m_rel / Wa
                            anP = psAG.tile([128, 2, 128], BF16, tag="ag")
                            for ft in range(2):
                                nc.tensor.transpose(
                                    anP[:, ft, :], an[:, ft * 128:(ft + 1) * 128],
                                    identb[:])
                            anT = sp.tile([128, 2, 128], BF16, tag="anT")
                            nc.vector.tensor_copy(anT[:], anP[:])
                            # m_rel block-diag transform (feature-major)
                            aggM = psAG.tile([128, 2, 128], F32, tag="ag")
                            for kt in range(2):
                                nc.tensor.matmul(aggM[:, kt, :], mblk_t[:, kt, :],
                                                 anT[:, kt, :], start=True, stop=True)
                            if r == 0:
                                nc.vector.tensor_copy(agg1T[:, :, w, :], aggM[:])
                                if DBG and l == 0 and w == 0:
                                    nc.sync.dma_start(dbg_an, an[:])
                                continue
                            # gelu (exact) in feature-major
                            geluT = sp.tile([128, 2, 128], BF16, tag="geluT")
                            if r == 2:
                                gin = sp.tile([128, 2, 128], F32, tag="gin")
                                nc.vector.tensor_add(gin[:], aggM[:], agg1T[:, :, w, :])
                                nc.scalar.activation(geluT[:], gin[:], AF.Gelu)
                            else:
                                nc.scalar.activation(geluT[:], aggM[:], AF.Gelu)
                            # Wa: node-major output from feature-major gelu
                            po = psPO.tile([128, H], F32, tag="po")
                            for kt in range(2):
                                nc.tensor.matmul(po[:], geluT[:, kt, :], wa_t[:, kt, :],
                                                 start=(kt == 0), stop=(kt == 1))
                            if ba_t is not None:
                                nc.vector.tensor_add(po[:], po[:], ba_t[:])
                            # h_pre = o + h (in place), s1 = row sums
                            nc.vector.scalar_tensor_tensor(
                                h[:, dt, w, :], po[:], 1.0, h[:, dt, w, :],
                                OP.mult, OP.add, accum_out=s1[:, w:w + 1])
                            sqs = sp.tile([128, H], F32, tag="sqs")
                            nc.scalar.activation(sqs[:], h[:, dt, w, :], AF.Square,
                                                 accum_out=s2[:, w:w + 1])
                    if DBG and l == 0 and r == 0:
                        nc.sync.dma_start(dbg_agg, agg1T[:])
                    if r != 0:
                        finish_type(dt, l)
                    nc.leave_named_scope(f"l{l}_r{r}", _sid, False)

                if DBG and l == 0:
                    nc.sync.dma_start(dbg_h, h[:])

            # ---- output projection ----
            with nc.named_scope("outproj"):
                wo = load_w(wout_d)
                bo = load_bias(bias_d["bout_b"]) if ub["bout"] else None
                for t in range(T):
                    for nt in range(NT):
                        hTt = transpose_tile(h[:, t, nt, :], nt)
                        ps = psPO.tile([128, OUT], F32, tag="po")
                        for kt in range(2):
                            nc.tensor.matmul(ps[:], hTt[:, kt, :], wo[:, kt, :OUT],
                                             start=(kt == 0), stop=(kt == 1))
                        st_ = stg.tile([128, OUT], F32, tag="yout")
                        if bo is not None:
                            nc.vector.tensor_add(st_[:], ps[:], bo[:, :OUT])
                        else:
                            nc.scalar.copy(st_[:], ps[:])
                        nc.sync.dma_start(y_d[t, nt * 128:(nt + 1) * 128, :], st_[:])
    nc.compile()
    return nc


def kernel(**inputs):
    shared, per_core, meta = _preprocess(inputs)
    shapes = {k: list(v.shape) for k, v in {**shared, **per_core[0]}.items()}
    nc = bacc.Bacc("TRN2", target_bir_lowering=False, debug=False, num_devices=NC)
    nc = _build(nc, meta, shapes)
    in_maps = [{**shared, **per_core[c]} for c in range(NC)]
    res = run_bass_kernel_spmd(nc, in_maps, core_ids=list(range(NC)))
    y = np.concatenate([res.results[c]["y"][:, :NL, :] for c in range(NC)], axis=1)
    return y.astype(np.float32)


if __name__ == "__main__":
    import reference
    inputs = {k: np.asarray(v) for k, v in reference.setup_inputs().items()}
    out = kernel(**inputs)
    exp = np.asarray(reference.reference(**inputs))
    err = np.abs(out - exp).max() / np.abs(exp).max()
    print("Relative error:", err)


# revision 39
# speedup vs baseline: 1.0479x; 1.0479x over previous
"""HGT link predictor on 8 trn2 NeuronCores.

Sharding: nodes split 8 ways per type (2500/core, padded to 2560).
Params replicated. Edges partitioned by destination core, sorted by dst,
packed into 128-edge chunks within 128-dst-node windows.

v2 design:
- All matmul operands bf16 (PSUM accumulates fp32): 4x PE throughput
  vs fp32 (4 cycles/row), and half the DMA/collective bytes.
- a_rel (scaled by p_rel/sqrt(d)) folded into per-relation q
  projections (dst side, local); m_rel applied post-aggregation as
  block-diagonal matmuls (dst side, local, commutes with the softmax
  average per head). The halo exchange then only carries RAW k/v per
  source type: two bf16 AllGathers per layer ([2560,512] ->
  [20480,512] each), 8x less wire than per-relation folded fp32
  tables.
- Scatter-add via one-hot matmuls ([dst,256 agg | 8 denom] per
  128-dst window, PSUM-accumulated over edge chunks); softmax
  denominator rides as 8 extra msg columns. Normalized aggregate is
  transposed per window (bf16 PE transpose) so m_rel and Wa matmuls
  consume it feature-major with no extra copies.
- Collective outputs use addr_space="Shared"; per-layer double
  buffers remove cross-layer WAR serialization, letting layer l+1's
  projections and AllGathers overlap layer l's edge phase.
Per layer: kv projections -> AllGather(t0), AllGather(t1) overlapped
with q' projections -> per-relation edge phase (gather kv/q' rows,
logits via mul+segmented reduce, exp, one-hot scatter matmuls,
normalize, m_rel, gelu, Wa, gated skip + residual + LayerNorm + relu).
"""
import math
import numpy as np

import concourse.bacc as bacc
import concourse.bass as bass
import concourse.mybir as mybir
import concourse.tile as tile
from concourse.bass_utils import run_bass_kernel_spmd
from concourse.library_config import mlp

F32 = mybir.dt.float32
BF16 = mybir.dt.bfloat16
I16 = mybir.dt.int16
AF = mybir.ActivationFunctionType
OP = mybir.AluOpType

T, R, L = 3, 4, 2
H, HEADS, D, FIN, OUT = 256, 8, 32, 128, 128
SRC_T = (0, 1, 1, 1)
DST_T = (1, 0, 1, 2)
LN_EPS = 1e-5
NC = 8
N = 20000
DBG = False
PROJ_ORDER = (0, 1, 2)
EDGE_ORDER = (0, 1, 3, 2)
NL = N // NC          # 2500 real local nodes per type
NT = 20               # node tiles of 128
NLP = NT * 128        # 2560 padded local nodes
NWIN = NT             # dst windows of 128 local nodes
GWIN = 2              # windows per gather group
KV_W = 2 * H          # 512: [k || v] columns of a kv-table row


def _block_diag(a):
    """a: [HEADS, D, D] -> [H, H] block diagonal."""
    out = np.zeros((H, H), np.float32)
    for h in range(HEADS):
        out[h * D:(h + 1) * D, h * D:(h + 1) * D] = a[h]
    return out


def _wrap_idx(idx):
    """idx [M] -> [128, M//16] int16 wrapped in 16 partitions, replicated."""
    m = idx.shape[0]
    assert m % 16 == 0
    w = np.zeros((128, m // 16), np.int16)
    w[:16] = idx.astype(np.int16).reshape(m // 16, 16).T
    for rep in range(1, 8):
        w[16 * rep:16 * rep + 16] = w[:16]
    return w


def _preprocess(inputs):
    x = np.asarray(inputs["x"], np.float32)
    edge_index = np.asarray(inputs["edge_index"])
    Win = np.asarray(inputs["Win"], np.float32)
    b_in = np.asarray(inputs["b_in"], np.float32)
    Wk = np.asarray(inputs["Wk"], np.float32); bk = np.asarray(inputs["bk"], np.float32)
    Wq = np.asarray(inputs["Wq"], np.float32); bq = np.asarray(inputs["bq"], np.float32)
    Wv = np.asarray(inputs["Wv"], np.float32); bv = np.asarray(inputs["bv"], np.float32)
    Wa = np.asarray(inputs["Wa"], np.float32); ba = np.asarray(inputs["ba"], np.float32)
    skip = np.asarray(inputs["skip"], np.float32)
    a_rel = np.asarray(inputs["a_rel"], np.float32)
    m_rel = np.asarray(inputs["m_rel"], np.float32)
    p_rel = np.asarray(inputs["p_rel"], np.float32)
    ln_g = np.asarray(inputs["ln_g"], np.float32)
    ln_b = np.asarray(inputs["ln_b"], np.float32)
    Wout = np.asarray(inputs["Wout"], np.float32)
    bout = np.asarray(inputs["bout"], np.float32)

    meta = {}
    inv_sqrt_d = 1.0 / math.sqrt(D)
    # fold a_rel (scaled) into dst-side q projections per relation
    wq_eff = np.zeros((L, R, H, H), np.float32)
    bq_eff = np.zeros((L, R, H), np.float32)
    # block-diag m_rel chunks for post-aggregation transform (lhsT layout)
    m_blk = np.zeros((L, R, 2, 128, 128), np.float32)
    for l in range(L):
        for r in range(R):
            dt = DST_T[r]
            at = _block_diag(np.transpose(a_rel[l, r], (0, 2, 1))
                             * (p_rel[l, r] * inv_sqrt_d)[:, None, None])
            wq_eff[l, r] = Wq[l, dt] @ at
            bq_eff[l, r] = bq[l, dt] @ at
            mb = _block_diag(m_rel[l, r])
            m_blk[l, r, 0] = mb[0:128, 0:128]
            m_blk[l, r, 1] = mb[128:256, 128:256]
    beta = 1.0 / (1.0 + np.exp(-skip))          # [L, T]
    g = beta / (2.0 - beta)
    wa_eff = Wa * g[:, :, None, None]
    ba_eff = ba * g[:, :, None]
    meta["eps_eff"] = (LN_EPS / (2.0 - beta) ** 2).tolist()

    meta["use_bias"] = dict(
        bin_=bool(np.any(b_in)), bq=bool(np.any(bq_eff)),
        bkv=bool(np.any(bk[:, :2])) or bool(np.any(bv[:, :2])),
        ba=bool(np.any(ba_eff)), bout=bool(np.any(bout)),
        lng=not np.allclose(ln_g, 1.0), lnb=bool(np.any(ln_b)),
    )

    def bcast(v):
        # [..., F] -> [..., 128, F]: per-feature vectors replicated across partitions
        return np.ascontiguousarray(
            np.broadcast_to(v[..., None, :], v.shape[:-1] + (128, v.shape[-1])))

    # edge partitioning ---------------------------------------------------
    win_edges = [[] for _ in range(NC)]   # [c][r][w] -> (src_rows, dst_loc)
    kch_need = 1
    for c in range(NC):
        rel = []
        for r in range(R):
            src = edge_index[r, 0].astype(np.int64)
            dst = edge_index[r, 1].astype(np.int64)
            sel = (dst // NL) == c
            s, d = src[sel], dst[sel] - c * NL
            o = np.argsort(d, kind="stable")
            s, d = s[o], d[o]
            wins = []
            for w in range(NWIN):
                m = (d // 128) == w
                sw, dw = s[m], d[m]
                kch_need = max(kch_need, (len(sw) + 127) // 128)
                wins.append((sw, dw))
            rel.append(wins)
        win_edges[c] = rel
    KCH = kch_need
    meta["KCH"] = KCH
    NCHUNK = NWIN * KCH
    NIDX_R = NCHUNK * 128

    per_core = []
    for c in range(NC):
        oh = np.zeros((R, NCHUNK, 128, 128), np.float32)
        kv_idx = np.zeros((R, NIDX_R), np.int64)
        qi_idx = np.zeros((R, NIDX_R), np.int64)
        for r in range(R):
            for w in range(NWIN):
                sw, dw = win_edges[c][r][w]
                ne = len(sw)
                base = w * KCH * 128
                # src node n (global) -> kv-table row (n//NL)*NLP + n%NL
                kv_idx[r, base:base + ne] = (sw // NL) * NLP + (sw % NL)
                qi_idx[r, base:base + ne] = dw
                ch = base // 128 + np.arange(ne) // 128
                oh[r, ch, np.arange(ne) % 128, dw - w * 128] = 1.0
        # partition-major one-hot: [R, 128(edge), NCHUNK, 128(col)]
        oh_pm = np.ascontiguousarray(oh.transpose(0, 2, 1, 3))
        xc = np.zeros((T, 128, NLP), np.float32)
        xc[:, :, :NL] = x[:, c * NL:(c + 1) * NL, :].transpose(0, 2, 1)
        per_core.append(dict(
            xT_h=_bf(xc),
            oh=_bf(oh_pm),
            kv_idx=np.stack([_wrap_idx(kv_idx[r]) for r in range(R)]),
            qi_idx=np.stack([_wrap_idx(qi_idx[r]) for r in range(R)]),
        ))

    shared = dict(
        win=_bf(Win),                                     # [3,128,256]
        wk=_bf(Wk[:, :2]), wv=_bf(Wv[:, :2]),             # [L,2,256,256]
        wq=_bf(wq_eff), wa=_bf(wa_eff),
        m_blk=_bf(m_blk),
        wout=_bf(Wout),
        ident=np.eye(128, dtype=np.float32),
        identb=_bf(np.eye(128, dtype=np.float32)),
        bin_b=bcast(b_in), bq_b=bcast(bq_eff),
        bk_b=bcast(bk[:, :2]), bv_b=bcast(bv[:, :2]),
        ba_b=bcast(ba_eff), bout_b=bcast(bout),
        lng_b=bcast(ln_g), lnb_b=bcast(ln_b),
    )
    return shared, per_core, meta


def _bf(a):
    import ml_dtypes
    return np.ascontiguousarray(a).astype(ml_dtypes.bfloat16)


def NIDX_R16(KCH):
    return NWIN * KCH * 128 // 16


def _build(nc, meta, shapes):
    KCH = meta["KCH"]
    NCHUNK = NWIN * KCH
    GC = GWIN * KCH                      # chunks per gather group
    NGRP = NWIN // GWIN
    ub = meta["use_bias"]
    eps_eff = meta["eps_eff"]

    def din(name, dt_):
        return nc.dram_tensor(name, shapes[name], dt_, kind="ExternalInput").ap()

    xT_h = din("xT_h", BF16); oh_d = din("oh", BF16)
    kv_idx_d = din("kv_idx", I16); qi_idx_d = din("qi_idx", I16)
    win_d = din("win", BF16)
    wk_d = din("wk", BF16); wv_d = din("wv", BF16)
    wq_d = din("wq", BF16); wa_d = din("wa", BF16)
    m_blk_d = din("m_blk", BF16)
    wout_d = din("wout", BF16)
    ident_d = din("ident", F32); identb_d = din("identb", BF16)
    bias_d = {k: din(k, F32) for k in
              ("bin_b", "bq_b", "bk_b", "bv_b", "ba_b", "bout_b", "lng_b", "lnb_b")}
    y_d = nc.dram_tensor("y", [T, NLP, OUT], F32, kind="ExternalOutput").ap()
    if DBG:
        GC_ = GWIN * meta["KCH"]
        dbg_kvg = nc.dram_tensor("dbg_kvg", [128, GC_, KV_W], BF16,
                                 kind="ExternalOutput").ap()
        dbg_qig = nc.dram_tensor("dbg_qig", [128, GC_, H], BF16,
                                 kind="ExternalOutput").ap()
        dbg_msg = nc.dram_tensor("dbg_msg", [128, GC_, H + HEADS], BF16,
                                 kind="ExternalOutput").ap()
        dbg_agg = nc.dram_tensor("dbg_agg", [128, 2, NWIN, 128], BF16,
                                 kind="ExternalOutput").ap()
        dbg_an = nc.dram_tensor("dbg_an", [128, H], BF16,
                                kind="ExternalOutput").ap()
        dbg_h = nc.dram_tensor("dbg_h", [128, T, NT, H], F32,
                               kind="ExternalOutput").ap()

    def bc32(ap2d):
        """[..., k] AP -> [..., k, 32] stride-0 broadcast AP."""
        return bass.AP(tensor=ap2d.tensor, offset=ap2d.offset,
                       ap=list(ap2d.ap) + [[0, D]])

    with tile.TileContext(nc) as tc:
        with (
            tc.tile_pool(name="persist", bufs=1) as pp,
            tc.tile_pool(name="wpool", bufs=8) as wp,
            tc.tile_pool(name="wsmall", bufs=3) as ws,
            tc.tile_pool(name="stage", bufs=2) as stg,
            tc.tile_pool(name="edge", bufs=3) as ep,
            tc.tile_pool(name="small", bufs=4) as sp,
            tc.tile_pool(name="psSC", bufs=2, space="PSUM") as psSC,
            tc.tile_pool(name="psAG", bufs=3, space="PSUM") as psAG,
            tc.tile_pool(name="psPO", bufs=3, space="PSUM") as psPO,
            tc.tile_pool(name="dram", bufs=1, space="DRAM") as dp,
        ):
            nc.gpsimd.load_library(mlp)

            ident = pp.tile([128, 128], F32, tag="ident")
            nc.sync.dma_start(ident[:], ident_d)
            identb = pp.tile([128, 128], BF16, tag="identb")
            nc.sync.dma_start(identb[:], identb_d)
            h = pp.tile([128, T, NT, H], F32, tag="h")
            agg1T = pp.tile([128, 2, NT, 128], BF16, tag="agg1T")

            kv_loc = [[dp.tile([NLP, KV_W], BF16, name=f"kv_loc{l}{t}")
                       for t in range(2)] for l in range(L)]
            kv_full = [[dp.tile([NC * NLP, KV_W], BF16, addr_space="Shared",
                                name=f"kv_full{l}{t}")
                        for t in range(2)] for l in range(L)]
            q_dram = [dp.tile([R, NLP, H], BF16, name=f"q_dram{l}")
                      for l in range(L)]

            def load_w(src_ap):
                """[256, M] bf16 dram -> [128, 2, M] sbuf tile."""
                m = src_ap.shape[-1]
                t_ = wp.tile([128, 2, m], BF16, tag="w")
                nc.sync.dma_start(t_[:], src_ap.rearrange("(kt kp) m -> kp kt m", kp=128))
                return t_

            def load_bias(src_ap):
                t_ = wp.tile([128, H], F32, tag="bias")
                nc.sync.dma_start(t_[:], src_ap)
                return t_

            # ---- input projection: h[t] = relu(xT^T @ Win + b) ----
            # (invoked per type from the layer-0 projection loop so the kv
            #  AllGathers launch as early as possible)
            def inproj_type(t):
                w_in = ws.tile([128, H], BF16, tag="win", name="w_in")
                nc.sync.dma_start(w_in[:], win_d[t])
                bt = load_bias(bias_d["bin_b"][t]) if ub["bin_"] else None
                for nt in range(NT):
                    xt = ws.tile([128, 128], BF16, tag="xt", name="xt")
                    nc.sync.dma_start(xt[:], xT_h[t, :, nt * 128:(nt + 1) * 128])
                    ps = psPO.tile([128, H], F32, tag="po", name="ps_in")
                    nc.tensor.matmul(ps[:], xt[:], w_in[:], start=True, stop=True)
                    if bt is not None:
                        nc.vector.tensor_add(ps[:], ps[:], bt[:])
                    nc.scalar.activation(h[:, t, nt, :], ps[:], AF.Relu)

            def transpose_tile(src2, nt_label):
                """h tile [128, 256] f32 -> hT [128, 2, 128] bf16 (feature-major)."""
                hTt = ws.tile([128, 2, 128], BF16, tag="hTt")
                for ft in range(2):
                    tp = psSC.tile([128, 128], F32, tag="sc")
                    nc.tensor.transpose(tp[:], src2[:, ft * 128:(ft + 1) * 128], ident[:])
                    if (nt_label + ft) % 2:
                        nc.vector.tensor_copy(hTt[:, ft, :], tp[:])
                    else:
                        nc.scalar.copy(hTt[:, ft, :], tp[:])
                return hTt

            for l in range(L):
                # ---- per-type transposes + projections ----
                # type order 1,0,2: h[1] finishes first in the previous
                # layer's edge phase (r order 0,2,1,3), so its kv AllGather
                # launches earliest and overlaps the remaining edge work.
                with nc.named_scope(f"l{l}_proj"):
                    for t in PROJ_ORDER:
                        if l == 0:
                            with nc.named_scope(f"inproj{t}"):
                                inproj_type(t)
                        projs = []   # (wtile, btile, kind, dest-info)
                        if t < 2:
                            wkt = load_w(wk_d[l, t]); wvt = load_w(wv_d[l, t])
                            bkt = load_bias(bias_d["bk_b"][l, t]) if ub["bkv"] else None
                            bvt = load_bias(bias_d["bv_b"][l, t]) if ub["bkv"] else None
                            projs.append((wkt, bkt, "kv", 0))
                            projs.append((wvt, bvt, "kv", H))
                        for r in range(R):
                            if DST_T[r] == t:
                                wqr = load_w(wq_d[l, r])
                                bqr = load_bias(bias_d["bq_b"][l, r]) if ub["bq"] else None
                                projs.append((wqr, bqr, "q", r))
                        # staging: kv rows [128, NT/2, 512]; q rows [128, NT/2, 256]
                        for half in range(2):
                            kvstg = stg.tile([128, NT // 2, KV_W], BF16, tag="kvstg",
                                             name="kvstg") if t < 2 else None
                            qstg = {r: stg.tile([128, NT // 2, H], BF16, tag="qstg",
                                                name=f"qstg{r}")
                                    for (_, _, kind, r) in projs if kind == "q"}
                            for nti in range(NT // 2):
                                nt = half * (NT // 2) + nti
                                hTt = transpose_tile(h[:, t, nt, :], nt)
                                for (wt, bt, kind, info) in projs:
                                    ps = psPO.tile([128, H], F32, tag="po")
                                    for kt in range(2):
                                        nc.tensor.matmul(ps[:], hTt[:, kt, :],
                                                         wt[:, kt, :],
                                                         start=(kt == 0), stop=(kt == 1))
                                    dst_ = kvstg[:, nti, info:info + H] if kind == "kv" \
                                        else qstg[info][:, nti, :]
                                    if bt is not None:
                                        nc.vector.tensor_add(dst_, ps[:], bt[:])
                                    else:
                                        nc.vector.tensor_copy(dst_, ps[:])
                            r0_ = half * (NT // 2) * 128
                            r1_ = r0_ + (NT // 2) * 128
                            if t < 2:
                                nc.sync.dma_start(
                                    kv_loc[l][t][r0_:r1_, :].rearrange(
                                        "(nt kp) m -> kp nt m", kp=128),
                                    kvstg[:])
                            for r, qs in qstg.items():
                                nc.sync.dma_start(
                                    q_dram[l][r, r0_:r1_, :].rearrange(
                                        "(nt kp) m -> kp nt m", kp=128),
                                    qs[:])
                        if t < 2:
                            with nc.named_scope(f"l{l}_ag{t}"):
                                nc.gpsimd.collective_compute(
                                    "AllGather", OP.bypass,
                                    replica_groups=[list(range(NC))],
                                    ins=[kv_loc[l][t][:].opt()],
                                    outs=[kv_full[l][t][:].opt()],
                                )

                # ---- edge phase; r order: 0 (t1 agg), 1 (t0), 3 (t2), 2 (t1) ----
                s1 = sp.tile([128, NT], F32, tag="s1")
                s2 = sp.tile([128, NT], F32, tag="s2")

                def finish_type(t, l):
                    mu = sp.tile([128, NT], F32, tag="mu")
                    inv = sp.tile([128, NT], F32, tag="inv")
                    nmi = sp.tile([128, NT], F32, tag="nmi")
                    nc.vector.tensor_scalar_mul(mu[:], s1[:], 1.0 / H)
                    nc.vector.tensor_scalar_mul(inv[:], s2[:], 1.0 / H)  # mean sq
                    musq = sp.tile([128, NT], F32, tag="musq")
                    nc.vector.tensor_mul(musq[:], mu[:], mu[:])
                    nc.vector.scalar_tensor_tensor(
                        inv[:], inv[:], float(eps_eff[l][t]), musq[:],
                        OP.add, OP.subtract)              # var + eps
                    nc.scalar.activation(inv[:], inv[:], AF.Sqrt)
                    nc.vector.reciprocal(inv[:], inv[:])
                    nc.vector.scalar_tensor_tensor(
                        nmi[:], mu[:], -1.0, inv[:], OP.mult, OP.mult)
                    if ub["lng"] or ub["lnb"]:
                        lng_t = load_bias(bias_d["lng_b"][l, t])
                        lnb_t = load_bias(bias_d["lnb_b"][l, t])
                        for w in range(NT):
                            nc.scalar.activation(
                                h[:, t, w, :], h[:, t, w, :], AF.Identity,
                                bias=nmi[:, w:w + 1], scale=inv[:, w:w + 1])
                            nc.vector.tensor_mul(h[:, t, w, :], h[:, t, w, :], lng_t[:])
                            nc.vector.tensor_add(h[:, t, w, :], h[:, t, w, :], lnb_t[:])
                            nc.scalar.activation(h[:, t, w, :], h[:, t, w, :], AF.Relu)
                    else:
                        for w in range(NT):
                            nc.scalar.activation(
                                h[:, t, w, :], h[:, t, w, :], AF.Relu,
                                bias=nmi[:, w:w + 1], scale=inv[:, w:w + 1])

                for r in EDGE_ORDER:
                    _sid, _ = nc.enter_named_scope(f"l{l}_r{r}", False)
                    dt = DST_T[r]
                    st = SRC_T[r]
                    wa_t = ba_t = None
                    if r != 0:
                        wa_t = load_w(wa_d[l, dt])
                        ba_t = load_bias(bias_d["ba_b"][l, dt]) if ub["ba"] else None
                    mblk_t = ws.tile([128, 2, 128], BF16, tag="mblk")
                    nc.sync.dma_start(mblk_t[:], m_blk_d[l, r].rearrange("kt p m -> p kt m"))
                    kvi = sp.tile([128, NIDX_R16(KCH)], I16, tag="kvi")
                    qii = sp.tile([128, NIDX_R16(KCH)], I16, tag="qii")
                    nc.gpsimd.dma_start(kvi[:], kv_idx_d[r])
                    nc.gpsimd.dma_start(qii[:], qi_idx_d[r])
                    for gidx in range(NGRP):
                        ni = GC * 128
                        kvg = ep.tile([128, GC, KV_W], BF16, tag="kvg")
                        qig = ep.tile([128, GC, H], BF16, tag="qig")
                        nc.gpsimd.dma_gather(
                            kvg[:], kv_full[l][st][:],
                            kvi[:, gidx * (ni // 16):(gidx + 1) * (ni // 16)],
                            ni, ni, KV_W)
                        nc.gpsimd.dma_gather(
                            qig[:], q_dram[l][r],
                            qii[:, gidx * (ni // 16):(gidx + 1) * (ni // 16)],
                            ni, ni, H)
                        ohg = ep.tile([128, GC, 128], BF16, tag="ohg")
                        nc.sync.dma_start(ohg[:], oh_d[r, :, gidx * GC:(gidx + 1) * GC, :])
                        msg = ep.tile([128, GC, H + HEADS], BF16, tag="msg")
                        if DBG and l == 0 and r == 0 and gidx == 0:
                            nc.sync.dma_start(dbg_kvg, kvg[:])
                            nc.sync.dma_start(dbg_qig, qig[:])
                        lg = sp.tile([128, GC, HEADS], F32, tag="lg")
                        # q*k product staged in msg[:, :, 0:H]; overwritten by
                        # the weighted-v below after the reduce consumes it
                        nc.vector.tensor_mul(msg[:, :, 0:H], qig[:], kvg[:, :, 0:H])
                        nc.vector.tensor_reduce(
                            lg[:], msg[:, :, 0:H].rearrange("p g (hh dd) -> p g hh dd", dd=D),
                            mybir.AxisListType.X, OP.add)
                        nc.scalar.activation(msg[:, :, H:H + HEADS], lg[:], AF.Exp)
                        nc.vector.tensor_mul(
                            msg[:, :, 0:H].rearrange("p g (hh dd) -> p g hh dd", dd=D),
                            kvg[:, :, H:2 * H].rearrange("p g (hh dd) -> p g hh dd", dd=D),
                            bc32(msg[:, :, H:H + HEADS]))
                        if DBG and l == 0 and r == 0 and gidx == 0:
                            nc.sync.dma_start(dbg_msg, msg[:])
                        for wi in range(GWIN):
                            w = gidx * GWIN + wi
                            # scatter: node-major [dst, 256 agg | 8 denom]
                            pw = psSC.tile([128, 264], F32, tag="sc")
                            for kc in range(KCH):
                                ch = wi * KCH + kc
                                nc.tensor.matmul(pw[:], ohg[:, ch, :], msg[:, ch, :],
                                                 start=(kc == 0), stop=(kc == KCH - 1))
                            # rec = 1/denom  [128 dst, 8] bf16
                            # +1e-30: degree-0 dst nodes have sum 0; keep 0*recip = 0
                            recf = sp.tile([128, HEADS], F32, tag="recf")
                            nc.vector.tensor_scalar_add(recf[:], pw[:, H:H + HEADS], 1e-30)
                            rec = sp.tile([128, HEADS], BF16, tag="rec")
                            with nc.allow_low_precision(reason="softmax recip to bf16"):
                                nc.vector.reciprocal(rec[:], recf[:])
                            # normalized node-major agg, bf16
                            an = sp.tile([128, H], BF16, tag="an")
                            nc.vector.tensor_mul(
                                an[:].rearrange("p (hh dd) -> p hh dd", dd=D),
                                pw[:, 0:H].rearrange("p (hh dd) -> p hh dd", dd=D),
                                bc32(rec[:]))
                            # transpose to feature-major for m_rel / Wa
                            anP = psAG.tile([128, 2, 128], BF16, tag="ag")
                            for ft in range(2):
                                nc.tensor.transpose(
                                    anP[:, ft, :], an[:, ft * 128:(ft + 1) * 128],
                                    identb[:])
                            anT = sp.tile([128, 2, 128], BF16, tag="anT")
                            nc.vector.tensor_copy(anT[:], anP[:])
                            # m_rel block-diag transform (feature-major)
                            aggM = psAG.tile([128, 2, 128], F32, tag="ag")
                            for kt in range(2):
                                nc.tensor.matmul(aggM[:, kt, :], mblk_t[:, kt, :],
                                                 anT[:, kt, :], start=True, stop=True)
                            if r == 0:
                                nc.vector.tensor_copy(agg1T[:, :, w, :], aggM[:])
                                if DBG and l == 0 and w == 0:
                                    nc.sync.dma_start(dbg_an, an[:])
                                continue
                            # gelu (exact) in feature-major
                            geluT = sp.tile([128, 2, 128], BF16, tag="geluT")
                            if r == 2:
                                gin = sp.tile([128, 2, 128], F32, tag="gin")
                                nc.vector.tensor_add(gin[:], aggM[:], agg1T[:, :, w, :])
                                nc.scalar.activation(geluT[:], gin[:], AF.Gelu)
                            else:
                                nc.scalar.activation(geluT[:], aggM[:], AF.Gelu)
                            # Wa: node-major output from feature-major gelu
                            po = psPO.tile([128, H], F32, tag="po")
                            for kt in range(2):
                                nc.tensor.matmul(po[:], geluT[:, kt, :], wa_t[:, kt, :],
                                                 start=(kt == 0), stop=(kt == 1))
                            if ba_t is not None:
                                nc.vector.tensor_add(po[:], po[:], ba_t[:])
                            # h_pre = o + h (in place), s1 = row sums
                            nc.vector.scalar_tensor_tensor(
                                h[:, dt, w, :], po[:], 1.0, h[:, dt, w, :],
                                OP.mult, OP.add, accum_out=s1[:, w:w + 1])
                            sqs = sp.tile([128, H], F32, tag="sqs")
                            nc.scalar.activation(sqs[:], h[:, dt, w, :], AF.Square,
                                                 accum_out=s2[:, w:w + 1])
                    if DBG and l == 0 and r == 0:
                        nc.sync.dma_start(dbg_agg, agg1T[:])
                    if r != 0:
                        finish_type(dt, l)
                    nc.leave_named_scope(f"l{l}_r{r}", _sid, False)

                if DBG and l == 0:
                    nc.sync.dma_start(dbg_h, h[:])

            # ---- output projection ----
            with nc.named_scope("outproj"):
                wo = load_w(wout_d)
                bo = load_bias(bias_d["bout_b"]) if ub["bout"] else None
                for t in range(T):
                    for nt in range(NT):
                        hTt = transpose_tile(h[:, t, nt, :], nt)
                        ps = psPO.tile([128, OUT], F32, tag="po")
                        for kt in range(2):
                            nc.tensor.matmul(ps[:], hTt[:, kt, :], wo[:, kt, :OUT],
                                             start=(kt == 0), stop=(kt == 1))
                        st_ = stg.tile([128, OUT], F32, tag="yout")
                        if bo is not None:
                            nc.vector.tensor_add(st_[:], ps[:], bo[:, :OUT])
                        else:
                            nc.scalar.copy(st_[:], ps[:])
                        nc.sync.dma_start(y_d[t, nt * 128:(nt + 1) * 128, :], st_[:])
    nc.compile()
    return nc


def kernel(**inputs):
    shared, per_core, meta = _preprocess(inputs)
    shapes = {k: list(v.shape) for k, v in {**shared, **per_core[0]}.items()}
    nc = bacc.Bacc("TRN2", target_bir_lowering=False, debug=False, num_devices=NC)
    nc = _build(nc, meta, shapes)
    in_maps = [{**shared, **per_core[c]} for c in range(NC)]
    res = run_bass_kernel_spmd(nc, in_maps, core_ids=list(range(NC)))
    y = np.concatenate([res.results[c]["y"][:, :NL, :] for c in range(NC)], axis=1)
    return y.astype(np.float32)


if __name__ == "__main__":
    import reference
    inputs = {k: np.asarray(v) for k, v in reference.setup_inputs().items()}
    out = kernel(**inputs)
    exp = np.asarray(reference.reference(**inputs))
    err = np.abs(out - exp).max() / np.abs(exp).max()
    print("Relative error:", err)


# revision 43
# speedup vs baseline: 1.0587x; 1.0103x over previous
"""HGT link predictor on 8 trn2 NeuronCores.

Sharding: nodes split 8 ways per type (2500/core, padded to 2560).
Params replicated. Edges partitioned by destination core, sorted by dst,
packed into 128-edge chunks within 128-dst-node windows.

v2 design:
- All matmul operands bf16 (PSUM accumulates fp32): 4x PE throughput
  vs fp32 (4 cycles/row), and half the DMA/collective bytes.
- a_rel (scaled by p_rel/sqrt(d)) folded into per-relation q
  projections (dst side, local); m_rel applied post-aggregation as
  block-diagonal matmuls (dst side, local, commutes with the softmax
  average per head). The halo exchange then only carries RAW k/v per
  source type: two bf16 AllGathers per layer ([2560,512] ->
  [20480,512] each), 8x less wire than per-relation folded fp32
  tables.
- Scatter-add via one-hot matmuls ([dst,256 agg | 8 denom] per
  128-dst window, PSUM-accumulated over edge chunks); softmax
  denominator rides as 8 extra msg columns. Normalized aggregate is
  transposed per window (bf16 PE transpose) so m_rel and Wa matmuls
  consume it feature-major with no extra copies.
- Collective outputs use addr_space="Shared"; per-layer double
  buffers remove cross-layer WAR serialization, letting layer l+1's
  projections and AllGathers overlap layer l's edge phase.
Per layer: kv projections -> AllGather(t0), AllGather(t1) overlapped
with q' projections -> per-relation edge phase (gather kv/q' rows,
logits via mul+segmented reduce, exp, one-hot scatter matmuls,
normalize, m_rel, gelu, Wa, gated skip + residual + LayerNorm + relu).
"""
import math
import numpy as np

import concourse.bacc as bacc
import concourse.bass as bass
import concourse.mybir as mybir
import concourse.tile as tile
from concourse.bass_utils import run_bass_kernel_spmd
from concourse.library_config import mlp

F32 = mybir.dt.float32
BF16 = mybir.dt.bfloat16
I16 = mybir.dt.int16
AF = mybir.ActivationFunctionType
OP = mybir.AluOpType

T, R, L = 3, 4, 2
H, HEADS, D, FIN, OUT = 256, 8, 32, 128, 128
SRC_T = (0, 1, 1, 1)
DST_T = (1, 0, 1, 2)
LN_EPS = 1e-5
NC = 8
N = 20000
DBG = False
PROJ_ORDER = (0, 1, 2)
EDGE_ORDER = (0, 1, 3, 2)
NL = N // NC          # 2500 real local nodes per type
NT = 20               # node tiles of 128
NLP = NT * 128        # 2560 padded local nodes
NWIN = NT             # dst windows of 128 local nodes
GWIN = 2              # windows per gather group
KV_W = 2 * H          # 512: [k || v] columns of a kv-table row


def _block_diag(a):
    """a: [HEADS, D, D] -> [H, H] block diagonal."""
    out = np.zeros((H, H), np.float32)
    for h in range(HEADS):
        out[h * D:(h + 1) * D, h * D:(h + 1) * D] = a[h]
    return out


def _wrap_idx(idx):
    """idx [M] -> [128, M//16] int16 wrapped in 16 partitions, replicated."""
    m = idx.shape[0]
    assert m % 16 == 0
    w = np.zeros((128, m // 16), np.int16)
    w[:16] = idx.astype(np.int16).reshape(m // 16, 16).T
    for rep in range(1, 8):
        w[16 * rep:16 * rep + 16] = w[:16]
    return w


def _preprocess(inputs):
    x = np.asarray(inputs["x"], np.float32)
    edge_index = np.asarray(inputs["edge_index"])
    Win = np.asarray(inputs["Win"], np.float32)
    b_in = np.asarray(inputs["b_in"], np.float32)
    Wk = np.asarray(inputs["Wk"], np.float32); bk = np.asarray(inputs["bk"], np.float32)
    Wq = np.asarray(inputs["Wq"], np.float32); bq = np.asarray(inputs["bq"], np.float32)
    Wv = np.asarray(inputs["Wv"], np.float32); bv = np.asarray(inputs["bv"], np.float32)
    Wa = np.asarray(inputs["Wa"], np.float32); ba = np.asarray(inputs["ba"], np.float32)
    skip = np.asarray(inputs["skip"], np.float32)
    a_rel = np.asarray(inputs["a_rel"], np.float32)
    m_rel = np.asarray(inputs["m_rel"], np.float32)
    p_rel = np.asarray(inputs["p_rel"], np.float32)
    ln_g = np.asarray(inputs["ln_g"], np.float32)
    ln_b = np.asarray(inputs["ln_b"], np.float32)
    Wout = np.asarray(inputs["Wout"], np.float32)
    bout = np.asarray(inputs["bout"], np.float32)

    meta = {}
    inv_sqrt_d = 1.0 / math.sqrt(D)
    # fold a_rel (scaled) into dst-side q projections per relation
    wq_eff = np.zeros((L, R, H, H), np.float32)
    bq_eff = np.zeros((L, R, H), np.float32)
    # block-diag m_rel chunks for post-aggregation transform (lhsT layout)
    m_blk = np.zeros((L, R, 2, 128, 128), np.float32)
    for l in range(L):
        for r in range(R):
            dt = DST_T[r]
            at = _block_diag(np.transpose(a_rel[l, r], (0, 2, 1))
                             * (p_rel[l, r] * inv_sqrt_d)[:, None, None])
            wq_eff[l, r] = Wq[l, dt] @ at
            bq_eff[l, r] = bq[l, dt] @ at
            mb = _block_diag(m_rel[l, r])
            m_blk[l, r, 0] = mb[0:128, 0:128]
            m_blk[l, r, 1] = mb[128:256, 128:256]
    beta = 1.0 / (1.0 + np.exp(-skip))          # [L, T]
    g = beta / (2.0 - beta)
    wa_eff = Wa * g[:, :, None, None]
    ba_eff = ba * g[:, :, None]
    meta["eps_eff"] = (LN_EPS / (2.0 - beta) ** 2).tolist()

    meta["use_bias"] = dict(
        bin_=bool(np.any(b_in)), bq=bool(np.any(bq_eff)),
        bkv=bool(np.any(bk[:, :2])) or bool(np.any(bv[:, :2])),
        ba=bool(np.any(ba_eff)), bout=bool(np.any(bout)),
        lng=not np.allclose(ln_g, 1.0), lnb=bool(np.any(ln_b)),
    )

    def bcast(v):
        # [..., F] -> [..., 128, F]: per-feature vectors replicated across partitions
        return np.ascontiguousarray(
            np.broadcast_to(v[..., None, :], v.shape[:-1] + (128, v.shape[-1])))

    # edge partitioning ---------------------------------------------------
    win_edges = [[] for _ in range(NC)]   # [c][r][w] -> (src_rows, dst_loc)
    kch_need = 1
    for c in range(NC):
        rel = []
        for r in range(R):
            src = edge_index[r, 0].astype(np.int64)
            dst = edge_index[r, 1].astype(np.int64)
            sel = (dst // NL) == c
            s, d = src[sel], dst[sel] - c * NL
            o = np.argsort(d, kind="stable")
            s, d = s[o], d[o]
            wins = []
            for w in range(NWIN):
                m = (d // 128) == w
                sw, dw = s[m], d[m]
                kch_need = max(kch_need, (len(sw) + 127) // 128)
                wins.append((sw, dw))
            rel.append(wins)
        win_edges[c] = rel
    KCH = kch_need
    meta["KCH"] = KCH
    NCHUNK = NWIN * KCH
    NIDX_R = NCHUNK * 128

    per_core = []
    for c in range(NC):
        oh = np.zeros((R, NCHUNK, 128, 128), np.float32)
        kv_idx = np.zeros((R, NIDX_R), np.int64)
        qi_idx = np.zeros((R, NIDX_R), np.int64)
        for r in range(R):
            for w in range(NWIN):
                sw, dw = win_edges[c][r][w]
                ne = len(sw)
                base = w * KCH * 128
                # src node n (global) -> kv-table row (n//NL)*NLP + n%NL
                kv_idx[r, base:base + ne] = (sw // NL) * NLP + (sw % NL)
                qi_idx[r, base:base + ne] = dw
                ch = base // 128 + np.arange(ne) // 128
                oh[r, ch, np.arange(ne) % 128, dw - w * 128] = 1.0
        # partition-major one-hot: [R, 128(edge), NCHUNK, 128(col)]
        oh_pm = np.ascontiguousarray(oh.transpose(0, 2, 1, 3))
        xc = np.zeros((T, 128, NLP), np.float32)
        xc[:, :, :NL] = x[:, c * NL:(c + 1) * NL, :].transpose(0, 2, 1)
        per_core.append(dict(
            xT_h=_bf(xc),
            oh=_bf(oh_pm),
            kv_idx=np.stack([_wrap_idx(kv_idx[r]) for r in range(R)]),
            qi_idx=np.stack([_wrap_idx(qi_idx[r]) for r in range(R)]),
        ))

    shared = dict(
        win=_bf(Win),                                     # [3,128,256]
        wk=_bf(Wk[:, :2]), wv=_bf(Wv[:, :2]),             # [L,2,256,256]
        wq=_bf(wq_eff), wa=_bf(wa_eff),
        m_blk=_bf(m_blk),
        wout=_bf(Wout),
        ident=np.eye(128, dtype=np.float32),
        identb=_bf(np.eye(128, dtype=np.float32)),
        bin_b=bcast(b_in), bq_b=bcast(bq_eff),
        bk_b=bcast(bk[:, :2]), bv_b=bcast(bv[:, :2]),
        ba_b=bcast(ba_eff), bout_b=bcast(bout),
        lng_b=bcast(ln_g), lnb_b=bcast(ln_b),
    )
    return shared, per_core, meta


def _bf(a):
    import ml_dtypes
    return np.ascontiguousarray(a).astype(ml_dtypes.bfloat16)


def NIDX_R16(KCH):
    return NWIN * KCH * 128 // 16


def _build(nc, meta, shapes):
    KCH = meta["KCH"]
    NCHUNK = NWIN * KCH
    GC = GWIN * KCH                      # chunks per gather group
    NGRP = NWIN // GWIN
    ub = meta["use_bias"]
    eps_eff = meta["eps_eff"]

    def din(name, dt_):
        return nc.dram_tensor(name, shapes[name], dt_, kind="ExternalInput").ap()

    xT_h = din("xT_h", BF16); oh_d = din("oh", BF16)
    kv_idx_d = din("kv_idx", I16); qi_idx_d = din("qi_idx", I16)
    win_d = din("win", BF16)
    wk_d = din("wk", BF16); wv_d = din("wv", BF16)
    wq_d = din("wq", BF16); wa_d = din("wa", BF16)
    m_blk_d = din("m_blk", BF16)
    wout_d = din("wout", BF16)
    ident_d = din("ident", F32); identb_d = din("identb", BF16)
    bias_d = {k: din(k, F32) for k in
              ("bin_b", "bq_b", "bk_b", "bv_b", "ba_b", "bout_b", "lng_b", "lnb_b")}
    y_d = nc.dram_tensor("y", [T, NLP, OUT], F32, kind="ExternalOutput").ap()
    if DBG:
        GC_ = GWIN * meta["KCH"]
        dbg_kvg = nc.dram_tensor("dbg_kvg", [128, GC_, KV_W], BF16,
                                 kind="ExternalOutput").ap()
        dbg_qig = nc.dram_tensor("dbg_qig", [128, GC_, H], BF16,
                                 kind="ExternalOutput").ap()
        dbg_msg = nc.dram_tensor("dbg_msg", [128, GC_, H + HEADS], BF16,
                                 kind="ExternalOutput").ap()
        dbg_agg = nc.dram_tensor("dbg_agg", [128, 2, NWIN, 128], BF16,
                                 kind="ExternalOutput").ap()
        dbg_an = nc.dram_tensor("dbg_an", [128, H], BF16,
                                kind="ExternalOutput").ap()
        dbg_h = nc.dram_tensor("dbg_h", [128, T, NT, H], F32,
                               kind="ExternalOutput").ap()

    def bc32(ap2d):
        """[..., k] AP -> [..., k, 32] stride-0 broadcast AP."""
        return bass.AP(tensor=ap2d.tensor, offset=ap2d.offset,
                       ap=list(ap2d.ap) + [[0, D]])

    with tile.TileContext(nc) as tc:
        with (
            tc.tile_pool(name="persist", bufs=1) as pp,
            tc.tile_pool(name="wpool", bufs=8) as wp,
            tc.tile_pool(name="wsmall", bufs=3) as ws,
            tc.tile_pool(name="stage", bufs=2) as stg,
            tc.tile_pool(name="edge", bufs=3) as ep,
            tc.tile_pool(name="small", bufs=3) as sp,
            tc.tile_pool(name="idx", bufs=2) as ip,
            tc.tile_pool(name="psSC", bufs=2, space="PSUM") as psSC,
            tc.tile_pool(name="psAG", bufs=3, space="PSUM") as psAG,
            tc.tile_pool(name="psPO", bufs=3, space="PSUM") as psPO,
            tc.tile_pool(name="dram", bufs=1, space="DRAM") as dp,
        ):
            nc.gpsimd.load_library(mlp)

            ident = pp.tile([128, 128], F32, tag="ident")
            nc.sync.dma_start(ident[:], ident_d)
            identb = pp.tile([128, 128], BF16, tag="identb")
            nc.sync.dma_start(identb[:], identb_d)
            h = pp.tile([128, T, NT, H], F32, tag="h")
            agg1T = pp.tile([128, 2, NT, 128], BF16, tag="agg1T")

            kv_loc = [[dp.tile([NLP, KV_W], BF16, name=f"kv_loc{l}{t}")
                       for t in range(2)] for l in range(L)]
            kv_full = [[dp.tile([NC * NLP, KV_W], BF16, addr_space="Shared",
                                name=f"kv_full{l}{t}")
                        for t in range(2)] for l in range(L)]
            q_dram = [dp.tile([R, NLP, H], BF16, name=f"q_dram{l}")
                      for l in range(L)]

            def load_w(src_ap):
                """[256, M] bf16 dram -> [128, 2, M] sbuf tile."""
                m = src_ap.shape[-1]
                t_ = wp.tile([128, 2, m], BF16, tag="w")
                nc.sync.dma_start(t_[:], src_ap.rearrange("(kt kp) m -> kp kt m", kp=128))
                return t_

            def load_bias(src_ap):
                t_ = wp.tile([128, H], F32, tag="bias")
                nc.sync.dma_start(t_[:], src_ap)
                return t_

            # ---- input projection: h[t] = relu(xT^T @ Win + b) ----
            # (invoked per type from the layer-0 projection loop so the kv
            #  AllGathers launch as early as possible)
            def inproj_type(t):
                w_in = ws.tile([128, H], BF16, tag="win", name="w_in")
                nc.sync.dma_start(w_in[:], win_d[t])
                bt = load_bias(bias_d["bin_b"][t]) if ub["bin_"] else None
                for nt in range(NT):
                    xt = ws.tile([128, 128], BF16, tag="xt", name="xt")
                    nc.sync.dma_start(xt[:], xT_h[t, :, nt * 128:(nt + 1) * 128])
                    ps = psPO.tile([128, H], F32, tag="po", name="ps_in")
                    nc.tensor.matmul(ps[:], xt[:], w_in[:], start=True, stop=True)
                    if bt is not None:
                        nc.vector.tensor_add(ps[:], ps[:], bt[:])
                    nc.scalar.activation(h[:, t, nt, :], ps[:], AF.Relu)

            def transpose_tile(src2, nt_label):
                """h tile [128, 256] f32 -> hT [128, 2, 128] bf16 (feature-major)."""
                hTt = ws.tile([128, 2, 128], BF16, tag="hTt")
                for ft in range(2):
                    tp = psSC.tile([128, 128], F32, tag="sc")
                    nc.tensor.transpose(tp[:], src2[:, ft * 128:(ft + 1) * 128], ident[:])
                    if (nt_label + ft) % 2:
                        nc.vector.tensor_copy(hTt[:, ft, :], tp[:])
                    else:
                        nc.scalar.copy(hTt[:, ft, :], tp[:])
                return hTt

            for l in range(L):
                # ---- projections ----
                # Pass 1: transposes + kv projections per src type, each
                # followed immediately by its AllGather so both collectives
                # are in flight before any q' work. hT for types 0/1 is kept
                # for pass 2; type 2 transposes inline.
                with nc.named_scope(f"l{l}_proj"):
                    hTbig = pp.tile([128, 2, 2, NT, 128], BF16, tag="hTbig",
                                    name="hTbig")
                    for t in (0, 1):
                        if l == 0:
                            with nc.named_scope(f"inproj{t}"):
                                inproj_type(t)
                        wkt = load_w(wk_d[l, t]); wvt = load_w(wv_d[l, t])
                        bkt = load_bias(bias_d["bk_b"][l, t]) if ub["bkv"] else None
                        bvt = load_bias(bias_d["bv_b"][l, t]) if ub["bkv"] else None
                        for half in range(2):
                            kvstg = stg.tile([128, NT // 2, KV_W], BF16, tag="kvstg",
                                             name="kvstg")
                            for nti in range(NT // 2):
                                nt = half * (NT // 2) + nti
                                for ft in range(2):
                                    tp = psSC.tile([128, 128], F32, tag="sc",
                                                   name="tp")
                                    nc.tensor.transpose(
                                        tp[:], h[:, t, nt, ft * 128:(ft + 1) * 128],
                                        ident[:])
                                    if (nt + ft) % 2:
                                        nc.vector.tensor_copy(hTbig[:, t, ft, nt, :], tp[:])
                                    else:
                                        nc.scalar.copy(hTbig[:, t, ft, nt, :], tp[:])
                                for (wt, bt, col) in ((wkt, bkt, 0), (wvt, bvt, H)):
                                    ps = psPO.tile([128, H], F32, tag="po", name="ps_kv")
                                    for kt in range(2):
                                        nc.tensor.matmul(ps[:], hTbig[:, t, kt, nt, :],
                                                         wt[:, kt, :],
                                                         start=(kt == 0), stop=(kt == 1))
                                    dst_ = kvstg[:, nti, col:col + H]
                                    if bt is not None:
                                        nc.vector.tensor_add(dst_, ps[:], bt[:])
                                    else:
                                        nc.vector.tensor_copy(dst_, ps[:])
                            r0_ = half * (NT // 2) * 128
                            r1_ = r0_ + (NT // 2) * 128
                            nc.sync.dma_start(
                                kv_loc[l][t][r0_:r1_, :].rearrange(
                                    "(nt kp) m -> kp nt m", kp=128),
                                kvstg[:])
                        with nc.named_scope(f"l{l}_ag{t}"):
                            nc.gpsimd.collective_compute(
                                "AllGather", OP.bypass,
                                replica_groups=[list(range(NC))],
                                ins=[kv_loc[l][t][:].opt()],
                                outs=[kv_full[l][t][:].opt()],
                            )
                    # Pass 2: q' projections per relation (edge order), from
                    # the retained hT (types 0/1) or inline transposes (t2).
                    if l == 0:
                        with nc.named_scope("inproj2"):
                            inproj_type(2)
                    for r in EDGE_ORDER:
                        dt_ = DST_T[r]
                        wqr = load_w(wq_d[l, r])
                        bqr = load_bias(bias_d["bq_b"][l, r]) if ub["bq"] else None
                        for half in range(2):
                            qstg = stg.tile([128, NT // 2, H], BF16, tag="qstg",
                                            name=f"qstg{r}")
                            for nti in range(NT // 2):
                                nt = half * (NT // 2) + nti
                                if dt_ < 2:
                                    hTt2 = hTbig[:, dt_, :, nt, :]
                                else:
                                    hTt = transpose_tile(h[:, dt_, nt, :], nt)
                                    hTt2 = hTt[:]
                                ps = psPO.tile([128, H], F32, tag="po", name="ps_q")
                                for kt in range(2):
                                    nc.tensor.matmul(ps[:], hTt2[:, kt, :],
                                                     wqr[:, kt, :],
                                                     start=(kt == 0), stop=(kt == 1))
                                if bqr is not None:
                                    nc.vector.tensor_add(qstg[:, nti, :], ps[:], bqr[:])
                                else:
                                    nc.vector.tensor_copy(qstg[:, nti, :], ps[:])
                            r0_ = half * (NT // 2) * 128
                            r1_ = r0_ + (NT // 2) * 128
                            nc.sync.dma_start(
                                q_dram[l][r, r0_:r1_, :].rearrange(
                                    "(nt kp) m -> kp nt m", kp=128),
                                qstg[:])

                # ---- edge phase; r order: 0 (t1 agg), 1 (t0), 3 (t2), 2 (t1) ----
                s1 = sp.tile([128, NT], F32, tag="s1")
                s2 = sp.tile([128, NT], F32, tag="s2")

                def finish_type(t, l):
                    mu = sp.tile([128, NT], F32, tag="mu")
                    inv = sp.tile([128, NT], F32, tag="inv")
                    nmi = sp.tile([128, NT], F32, tag="nmi")
                    nc.vector.tensor_scalar_mul(mu[:], s1[:], 1.0 / H)
                    nc.vector.tensor_scalar_mul(inv[:], s2[:], 1.0 / H)  # mean sq
                    musq = sp.tile([128, NT], F32, tag="musq")
                    nc.vector.tensor_mul(musq[:], mu[:], mu[:])
                    nc.vector.scalar_tensor_tensor(
                        inv[:], inv[:], float(eps_eff[l][t]), musq[:],
                        OP.add, OP.subtract)              # var + eps
                    nc.scalar.activation(inv[:], inv[:], AF.Sqrt)
                    nc.vector.reciprocal(inv[:], inv[:])
                    nc.vector.scalar_tensor_tensor(
                        nmi[:], mu[:], -1.0, inv[:], OP.mult, OP.mult)
                    if ub["lng"] or ub["lnb"]:
                        lng_t = load_bias(bias_d["lng_b"][l, t])
                        lnb_t = load_bias(bias_d["lnb_b"][l, t])
                        for w in range(NT):
                            nc.scalar.activation(
                                h[:, t, w, :], h[:, t, w, :], AF.Identity,
                                bias=nmi[:, w:w + 1], scale=inv[:, w:w + 1])
                            nc.vector.tensor_mul(h[:, t, w, :], h[:, t, w, :], lng_t[:])
                            nc.vector.tensor_add(h[:, t, w, :], h[:, t, w, :], lnb_t[:])
                            nc.scalar.activation(h[:, t, w, :], h[:, t, w, :], AF.Relu)
                    else:
                        for w in range(NT):
                            nc.scalar.activation(
                                h[:, t, w, :], h[:, t, w, :], AF.Relu,
                                bias=nmi[:, w:w + 1], scale=inv[:, w:w + 1])

                for r in EDGE_ORDER:
                    _sid, _ = nc.enter_named_scope(f"l{l}_r{r}", False)
                    dt = DST_T[r]
                    st = SRC_T[r]
                    wa_t = ba_t = None
                    if r != 0:
                        wa_t = load_w(wa_d[l, dt])
                        ba_t = load_bias(bias_d["ba_b"][l, dt]) if ub["ba"] else None
                    mblk_t = ws.tile([128, 2, 128], BF16, tag="mblk")
                    nc.sync.dma_start(mblk_t[:], m_blk_d[l, r].rearrange("kt p m -> p kt m"))
                    kvi = ip.tile([128, NIDX_R16(KCH)], I16, tag="kvi")
                    qii = ip.tile([128, NIDX_R16(KCH)], I16, tag="qii")
                    nc.gpsimd.dma_start(kvi[:], kv_idx_d[r])
                    nc.gpsimd.dma_start(qii[:], qi_idx_d[r])
                    for gidx in range(NGRP):
                        ni = GC * 128
                        kvg = ep.tile([128, GC, KV_W], BF16, tag="kvg")
                        qig = ep.tile([128, GC, H], BF16, tag="qig")
                        nc.gpsimd.dma_gather(
                            kvg[:], kv_full[l][st][:],
                            kvi[:, gidx * (ni // 16):(gidx + 1) * (ni // 16)],
                            ni, ni, KV_W)
                        nc.gpsimd.dma_gather(
                            qig[:], q_dram[l][r],
                            qii[:, gidx * (ni // 16):(gidx + 1) * (ni // 16)],
                            ni, ni, H)
                        ohg = ep.tile([128, GC, 128], BF16, tag="ohg")
                        nc.sync.dma_start(ohg[:], oh_d[r, :, gidx * GC:(gidx + 1) * GC, :])
                        msg = ep.tile([128, GC, H + HEADS], BF16, tag="msg")
                        if DBG and l == 0 and r == 0 and gidx == 0:
                            nc.sync.dma_start(dbg_kvg, kvg[:])
                            nc.sync.dma_start(dbg_qig, qig[:])
                        lg = sp.tile([128, GC, HEADS], F32, tag="lg")
                        # q*k product staged in msg[:, :, 0:H]; overwritten by
                        # the weighted-v below after the reduce consumes it
                        nc.vector.tensor_mul(msg[:, :, 0:H], qig[:], kvg[:, :, 0:H])
                        nc.vector.tensor_reduce(
                            lg[:], msg[:, :, 0:H].rearrange("p g (hh dd) -> p g hh dd", dd=D),
                            mybir.AxisListType.X, OP.add)
                        nc.scalar.activation(msg[:, :, H:H + HEADS], lg[:], AF.Exp)
                        nc.vector.tensor_mul(
                            msg[:, :, 0:H].rearrange("p g (hh dd) -> p g hh dd", dd=D),
                            kvg[:, :, H:2 * H].rearrange("p g (hh dd) -> p g hh dd", dd=D),
                            bc32(msg[:, :, H:H + HEADS]))
                        if DBG and l == 0 and r == 0 and gidx == 0:
                            nc.sync.dma_start(dbg_msg, msg[:])
                        for wi in range(GWIN):
                            w = gidx * GWIN + wi
                            # scatter: node-major [dst, 256 agg | 8 denom]
                            pw = psSC.tile([128, 264], F32, tag="sc")
                            for kc in range(KCH):
                                ch = wi * KCH + kc
                                nc.tensor.matmul(pw[:], ohg[:, ch, :], msg[:, ch, :],
                                                 start=(kc == 0), stop=(kc == KCH - 1))
                            # rec = 1/denom  [128 dst, 8] bf16
                            # +1e-30: degree-0 dst nodes have sum 0; keep 0*recip = 0
                            recf = sp.tile([128, HEADS], F32, tag="recf")
                            nc.vector.tensor_scalar_add(recf[:], pw[:, H:H + HEADS], 1e-30)
                            rec = sp.tile([128, HEADS], BF16, tag="rec")
                            with nc.allow_low_precision(reason="softmax recip to bf16"):
                                nc.vector.reciprocal(rec[:], recf[:])
                            # normalized node-major agg, bf16
                            an = sp.tile([128, H], BF16, tag="an")
                            nc.vector.tensor_mul(
                                an[:].rearrange("p (hh dd) -> p hh dd", dd=D),
                                pw[:, 0:H].rearrange("p (hh dd) -> p hh dd", dd=D),
                                bc32(rec[:]))
                            # transpose to feature-major for m_rel / Wa
                            anP = psAG.tile([128, 2, 128], BF16, tag="ag")
                            for ft in range(2):
                                nc.tensor.transpose(
                                    anP[:, ft, :], an[:, ft * 128:(ft + 1) * 128],
                                    identb[:])
                            anT = sp.tile([128, 2, 128], BF16, tag="anT")
                            nc.vector.tensor_copy(anT[:], anP[:])
                            # m_rel block-diag transform (feature-major)
                            aggM = psAG.tile([128, 2, 128], F32, tag="ag")
                            for kt in range(2):
                                nc.tensor.matmul(aggM[:, kt, :], mblk_t[:, kt, :],
                                                 anT[:, kt, :], start=True, stop=True)
                            if r == 0:
                                nc.vector.tensor_copy(agg1T[:, :, w, :], aggM[:])
                                if DBG and l == 0 and w == 0:
                                    nc.sync.dma_start(dbg_an, an[:])
                                continue
                            # gelu (exact) in feature-major
                            geluT = sp.tile([128, 2, 128], BF16, tag="geluT")
                            if r == 2:
                                gin = sp.tile([128, 2, 128], F32, tag="gin")
                                nc.vector.tensor_add(gin[:], aggM[:], agg1T[:, :, w, :])
                                nc.scalar.activation(geluT[:], gin[:], AF.Gelu)
                            else:
                                nc.scalar.activation(geluT[:], aggM[:], AF.Gelu)
                            # Wa: node-major output from feature-major gelu
                            po = psPO.tile([128, H], F32, tag="po")
                            for kt in range(2):
                                nc.tensor.matmul(po[:], geluT[:, kt, :], wa_t[:, kt, :],
                                                 start=(kt == 0), stop=(kt == 1))
                            if ba_t is not None:
                                nc.vector.tensor_add(po[:], po[:], ba_t[:])
                            # h_pre = o + h (in place), s1 = row sums
                            nc.vector.scalar_tensor_tensor(
                                h[:, dt, w, :], po[:], 1.0, h[:, dt, w, :],
                                OP.mult, OP.add, accum_out=s1[:, w:w + 1])
                            sqs = sp.tile([128, H], F32, tag="sqs")
                            nc.scalar.activation(sqs[:], h[:, dt, w, :], AF.Square,
                                                 accum_out=s2[:, w:w + 1])
                    if DBG and l == 0 and r == 0:
                        nc.sync.dma_start(dbg_agg, agg1T[:])
                    if r != 0:
                        finish_type(dt, l)
                    nc.leave_named_scope(f"l{l}_r{r}", _sid, False)

                if DBG and l == 0:
                    nc.sync.dma_start(dbg_h, h[:])

            # ---- output projection ----
            with nc.named_scope("outproj"):
                wo = load_w(wout_d)
                bo = load_bias(bias_d["bout_b"]) if ub["bout"] else None
                for t in range(T):
                    for nt in range(NT):
                        hTt = transpose_tile(h[:, t, nt, :], nt)
                        ps = psPO.tile([128, OUT], F32, tag="po")
                        for kt in range(2):
                            nc.tensor.matmul(ps[:], hTt[:, kt, :], wo[:, kt, :OUT],
                                             start=(kt == 0), stop=(kt == 1))
                        st_ = stg.tile([128, OUT], F32, tag="yout")
                        if bo is not None:
                            nc.vector.tensor_add(st_[:], ps[:], bo[:, :OUT])
                        else:
                            nc.scalar.copy(st_[:], ps[:])
                        nc.sync.dma_start(y_d[t, nt * 128:(nt + 1) * 128, :], st_[:])
    nc.compile()
    return nc


def kernel(**inputs):
    shared, per_core, meta = _preprocess(inputs)
    shapes = {k: list(v.shape) for k, v in {**shared, **per_core[0]}.items()}
    nc = bacc.Bacc("TRN2", target_bir_lowering=False, debug=False, num_devices=NC)
    nc = _build(nc, meta, shapes)
    in_maps = [{**shared, **per_core[c]} for c in range(NC)]
    res = run_bass_kernel_spmd(nc, in_maps, core_ids=list(range(NC)))
    y = np.concatenate([res.results[c]["y"][:, :NL, :] for c in range(NC)], axis=1)
    return y.astype(np.float32)


if __name__ == "__main__":
    import reference
    inputs = {k: np.asarray(v) for k, v in reference.setup_inputs().items()}
    out = kernel(**inputs)
    exp = np.asarray(reference.reference(**inputs))
    err = np.abs(out - exp).max() / np.abs(exp).max()
    print("Relative error:", err)


# revision 44
# speedup vs baseline: 1.0618x; 1.0029x over previous
"""HGT link predictor on 8 trn2 NeuronCores.

Sharding: nodes split 8 ways per type (2500/core, padded to 2560).
Params replicated. Edges partitioned by destination core, sorted by dst,
packed into 128-edge chunks within 128-dst-node windows.

v2 design:
- All matmul operands bf16 (PSUM accumulates fp32): 4x PE throughput
  vs fp32 (4 cycles/row), and half the DMA/collective bytes.
- a_rel (scaled by p_rel/sqrt(d)) folded into per-relation q
  projections (dst side, local); m_rel applied post-aggregation as
  block-diagonal matmuls (dst side, local, commutes with the softmax
  average per head). The halo exchange then only carries RAW k/v per
  source type: two bf16 AllGathers per layer ([2560,512] ->
  [20480,512] each), 8x less wire than per-relation folded fp32
  tables.
- Scatter-add via one-hot matmuls ([dst,256 agg | 8 denom] per
  128-dst window, PSUM-accumulated over edge chunks); softmax
  denominator rides as 8 extra msg columns. Normalized aggregate is
  transposed per window (bf16 PE transpose) so m_rel and Wa matmuls
  consume it feature-major with no extra copies.
- Collective outputs use addr_space="Shared"; per-layer double
  buffers remove cross-layer WAR serialization, letting layer l+1's
  projections and AllGathers overlap layer l's edge phase.
Per layer: kv projections -> AllGather(t0), AllGather(t1) overlapped
with q' projections -> per-relation edge phase (gather kv/q' rows,
logits via mul+segmented reduce, exp, one-hot scatter matmuls,
normalize, m_rel, gelu, Wa, gated skip + residual + LayerNorm + relu).
"""
import math
import numpy as np

import concourse.bacc as bacc
import concourse.bass as bass
import concourse.mybir as mybir
import concourse.tile as tile
from concourse.bass_utils import run_bass_kernel_spmd
from concourse.library_config import mlp

F32 = mybir.dt.float32
BF16 = mybir.dt.bfloat16
I16 = mybir.dt.int16
AF = mybir.ActivationFunctionType
OP = mybir.AluOpType

T, R, L = 3, 4, 2
H, HEADS, D, FIN, OUT = 256, 8, 32, 128, 128
SRC_T = (0, 1, 1, 1)
DST_T = (1, 0, 1, 2)
LN_EPS = 1e-5
NC = 8
N = 20000
DBG = False
PROJ_ORDER = (0, 1, 2)
EDGE_ORDER = (0, 1, 3, 2)
NL = N // NC          # 2500 real local nodes per type
NT = 20               # node tiles of 128
NLP = NT * 128        # 2560 padded local nodes
NWIN = NT             # dst windows of 128 local nodes
GWIN = 2              # windows per gather group
KV_W = 2 * H          # 512: [k || v] columns of a kv-table row


def _block_diag(a):
    """a: [HEADS, D, D] -> [H, H] block diagonal."""
    out = np.zeros((H, H), np.float32)
    for h in range(HEADS):
        out[h * D:(h + 1) * D, h * D:(h + 1) * D] = a[h]
    return out


def _wrap_idx(idx):
    """idx [M] -> [128, M//16] int16 wrapped in 16 partitions, replicated."""
    m = idx.shape[0]
    assert m % 16 == 0
    w = np.zeros((128, m // 16), np.int16)
    w[:16] = idx.astype(np.int16).reshape(m // 16, 16).T
    for rep in range(1, 8):
        w[16 * rep:16 * rep + 16] = w[:16]
    return w


def _preprocess(inputs):
    x = np.asarray(inputs["x"], np.float32)
    edge_index = np.asarray(inputs["edge_index"])
    Win = np.asarray(inputs["Win"], np.float32)
    b_in = np.asarray(inputs["b_in"], np.float32)
    Wk = np.asarray(inputs["Wk"], np.float32); bk = np.asarray(inputs["bk"], np.float32)
    Wq = np.asarray(inputs["Wq"], np.float32); bq = np.asarray(inputs["bq"], np.float32)
    Wv = np.asarray(inputs["Wv"], np.float32); bv = np.asarray(inputs["bv"], np.float32)
    Wa = np.asarray(inputs["Wa"], np.float32); ba = np.asarray(inputs["ba"], np.float32)
    skip = np.asarray(inputs["skip"], np.float32)
    a_rel = np.asarray(inputs["a_rel"], np.float32)
    m_rel = np.asarray(inputs["m_rel"], np.float32)
    p_rel = np.asarray(inputs["p_rel"], np.float32)
    ln_g = np.asarray(inputs["ln_g"], np.float32)
    ln_b = np.asarray(inputs["ln_b"], np.float32)
    Wout = np.asarray(inputs["Wout"], np.float32)
    bout = np.asarray(inputs["bout"], np.float32)

    meta = {}
    inv_sqrt_d = 1.0 / math.sqrt(D)
    # fold a_rel (scaled) into dst-side q projections per relation
    wq_eff = np.zeros((L, R, H, H), np.float32)
    bq_eff = np.zeros((L, R, H), np.float32)
    # block-diag m_rel chunks for post-aggregation transform (lhsT layout)
    m_blk = np.zeros((L, R, 2, 128, 128), np.float32)
    for l in range(L):
        for r in range(R):
            dt = DST_T[r]
            at = _block_diag(np.transpose(a_rel[l, r], (0, 2, 1))
                             * (p_rel[l, r] * inv_sqrt_d)[:, None, None])
            wq_eff[l, r] = Wq[l, dt] @ at
            bq_eff[l, r] = bq[l, dt] @ at
            mb = _block_diag(m_rel[l, r])
            m_blk[l, r, 0] = mb[0:128, 0:128]
            m_blk[l, r, 1] = mb[128:256, 128:256]
    beta = 1.0 / (1.0 + np.exp(-skip))          # [L, T]
    g = beta / (2.0 - beta)
    wa_eff = Wa * g[:, :, None, None]
    ba_eff = ba * g[:, :, None]
    meta["eps_eff"] = (LN_EPS / (2.0 - beta) ** 2).tolist()

    meta["use_bias"] = dict(
        bin_=bool(np.any(b_in)), bq=bool(np.any(bq_eff)),
        bkv=bool(np.any(bk[:, :2])) or bool(np.any(bv[:, :2])),
        ba=bool(np.any(ba_eff)), bout=bool(np.any(bout)),
        lng=not np.allclose(ln_g, 1.0), lnb=bool(np.any(ln_b)),
    )

    def bcast(v):
        # [..., F] -> [..., 128, F]: per-feature vectors replicated across partitions
        return np.ascontiguousarray(
            np.broadcast_to(v[..., None, :], v.shape[:-1] + (128, v.shape[-1])))

    # edge partitioning ---------------------------------------------------
    win_edges = [[] for _ in range(NC)]   # [c][r][w] -> (src_rows, dst_loc)
    kch_need = 1
    for c in range(NC):
        rel = []
        for r in range(R):
            src = edge_index[r, 0].astype(np.int64)
            dst = edge_index[r, 1].astype(np.int64)
            sel = (dst // NL) == c
            s, d = src[sel], dst[sel] - c * NL
            o = np.argsort(d, kind="stable")
            s, d = s[o], d[o]
            wins = []
            for w in range(NWIN):
                m = (d // 128) == w
                sw, dw = s[m], d[m]
                kch_need = max(kch_need, (len(sw) + 127) // 128)
                wins.append((sw, dw))
            rel.append(wins)
        win_edges[c] = rel
    KCH = kch_need
    meta["KCH"] = KCH
    NCHUNK = NWIN * KCH
    NIDX_R = NCHUNK * 128

    per_core = []
    for c in range(NC):
        oh = np.zeros((R, NCHUNK, 128, 128), np.float32)
        kv_idx = np.zeros((R, NIDX_R), np.int64)
        qi_idx = np.zeros((R, NIDX_R), np.int64)
        for r in range(R):
            for w in range(NWIN):
                sw, dw = win_edges[c][r][w]
                ne = len(sw)
                base = w * KCH * 128
                # src node n (global) -> kv-table row (n//NL)*NLP + n%NL
                kv_idx[r, base:base + ne] = (sw // NL) * NLP + (sw % NL)
                qi_idx[r, base:base + ne] = dw
                ch = base // 128 + np.arange(ne) // 128
                oh[r, ch, np.arange(ne) % 128, dw - w * 128] = 1.0
        # partition-major one-hot: [R, 128(edge), NCHUNK, 128(col)]
        oh_pm = np.ascontiguousarray(oh.transpose(0, 2, 1, 3))
        xc = np.zeros((T, 128, NLP), np.float32)
        xc[:, :, :NL] = x[:, c * NL:(c + 1) * NL, :].transpose(0, 2, 1)
        per_core.append(dict(
            xT_h=_bf(xc),
            oh=_bf(oh_pm),
            kv_idx=np.stack([_wrap_idx(kv_idx[r]) for r in range(R)]),
            qi_idx=np.stack([_wrap_idx(qi_idx[r]) for r in range(R)]),
        ))

    shared = dict(
        win=_bf(Win),                                     # [3,128,256]
        wk=_bf(Wk[:, :2]), wv=_bf(Wv[:, :2]),             # [L,2,256,256]
        wq=_bf(wq_eff), wa=_bf(wa_eff),
        m_blk=_bf(m_blk),
        wout=_bf(Wout),
        ident=np.eye(128, dtype=np.float32),
        identb=_bf(np.eye(128, dtype=np.float32)),
        bin_b=bcast(b_in), bq_b=bcast(bq_eff),
        bk_b=bcast(bk[:, :2]), bv_b=bcast(bv[:, :2]),
        ba_b=bcast(ba_eff), bout_b=bcast(bout),
        lng_b=bcast(ln_g), lnb_b=bcast(ln_b),
    )
    return shared, per_core, meta


def _bf(a):
    import ml_dtypes
    return np.ascontiguousarray(a).astype(ml_dtypes.bfloat16)


def NIDX_R16(KCH):
    return NWIN * KCH * 128 // 16


def _build(nc, meta, shapes):
    KCH = meta["KCH"]
    NCHUNK = NWIN * KCH
    GC = GWIN * KCH                      # chunks per gather group
    NGRP = NWIN // GWIN
    ub = meta["use_bias"]
    eps_eff = meta["eps_eff"]

    def din(name, dt_):
        return nc.dram_tensor(name, shapes[name], dt_, kind="ExternalInput").ap()

    xT_h = din("xT_h", BF16); oh_d = din("oh", BF16)
    kv_idx_d = din("kv_idx", I16); qi_idx_d = din("qi_idx", I16)
    win_d = din("win", BF16)
    wk_d = din("wk", BF16); wv_d = din("wv", BF16)
    wq_d = din("wq", BF16); wa_d = din("wa", BF16)
    m_blk_d = din("m_blk", BF16)
    wout_d = din("wout", BF16)
    ident_d = din("ident", F32); identb_d = din("identb", BF16)
    bias_d = {k: din(k, F32) for k in
              ("bin_b", "bq_b", "bk_b", "bv_b", "ba_b", "bout_b", "lng_b", "lnb_b")}
    y_d = nc.dram_tensor("y", [T, NLP, OUT], F32, kind="ExternalOutput").ap()
    if DBG:
        GC_ = GWIN * meta["KCH"]
        dbg_kvg = nc.dram_tensor("dbg_kvg", [128, GC_, KV_W], BF16,
                                 kind="ExternalOutput").ap()
        dbg_qig = nc.dram_tensor("dbg_qig", [128, GC_, H], BF16,
                                 kind="ExternalOutput").ap()
        dbg_msg = nc.dram_tensor("dbg_msg", [128, GC_, H + HEADS], BF16,
                                 kind="ExternalOutput").ap()
        dbg_agg = nc.dram_tensor("dbg_agg", [128, 2, NWIN, 128], BF16,
                                 kind="ExternalOutput").ap()
        dbg_an = nc.dram_tensor("dbg_an", [128, H], BF16,
                                kind="ExternalOutput").ap()
        dbg_h = nc.dram_tensor("dbg_h", [128, T, NT, H], F32,
                               kind="ExternalOutput").ap()

    def bc32(ap2d):
        """[..., k] AP -> [..., k, 32] stride-0 broadcast AP."""
        return bass.AP(tensor=ap2d.tensor, offset=ap2d.offset,
                       ap=list(ap2d.ap) + [[0, D]])

    with tile.TileContext(nc) as tc:
        with (
            tc.tile_pool(name="persist", bufs=1) as pp,
            tc.tile_pool(name="wpool", bufs=8) as wp,
            tc.tile_pool(name="wsmall", bufs=3) as ws,
            tc.tile_pool(name="stage", bufs=2) as stg,
            tc.tile_pool(name="edge", bufs=3) as ep,
            tc.tile_pool(name="small", bufs=3) as sp,
            tc.tile_pool(name="idx", bufs=2) as ip,
            tc.tile_pool(name="psSC", bufs=3, space="PSUM") as psSC,
            tc.tile_pool(name="psAG", bufs=2, space="PSUM") as psAG,
            tc.tile_pool(name="psPO", bufs=3, space="PSUM") as psPO,
            tc.tile_pool(name="dram", bufs=1, space="DRAM") as dp,
        ):
            nc.gpsimd.load_library(mlp)

            ident = pp.tile([128, 128], F32, tag="ident")
            nc.sync.dma_start(ident[:], ident_d)
            identb = pp.tile([128, 128], BF16, tag="identb")
            nc.sync.dma_start(identb[:], identb_d)
            h = pp.tile([128, T, NT, H], F32, tag="h")
            agg1T = pp.tile([128, 2, NT, 128], BF16, tag="agg1T")

            kv_loc = [[dp.tile([NLP, KV_W], BF16, name=f"kv_loc{l}{t}")
                       for t in range(2)] for l in range(L)]
            kv_full = [[dp.tile([NC * NLP, KV_W], BF16, addr_space="Shared",
                                name=f"kv_full{l}{t}")
                        for t in range(2)] for l in range(L)]
            q_dram = [dp.tile([R, NLP, H], BF16, name=f"q_dram{l}")
                      for l in range(L)]

            def load_w(src_ap):
                """[256, M] bf16 dram -> [128, 2, M] sbuf tile."""
                m = src_ap.shape[-1]
                t_ = wp.tile([128, 2, m], BF16, tag="w")
                nc.sync.dma_start(t_[:], src_ap.rearrange("(kt kp) m -> kp kt m", kp=128))
                return t_

            def load_bias(src_ap):
                t_ = wp.tile([128, H], F32, tag="bias")
                nc.sync.dma_start(t_[:], src_ap)
                return t_

            # ---- input projection: h[t] = relu(xT^T @ Win + b) ----
            # (invoked per type from the layer-0 projection loop so the kv
            #  AllGathers launch as early as possible)
            def inproj_type(t):
                w_in = ws.tile([128, H], BF16, tag="win", name="w_in")
                nc.sync.dma_start(w_in[:], win_d[t])
                bt = load_bias(bias_d["bin_b"][t]) if ub["bin_"] else None
                for nt in range(NT):
                    xt = ws.tile([128, 128], BF16, tag="xt", name="xt")
                    nc.sync.dma_start(xt[:], xT_h[t, :, nt * 128:(nt + 1) * 128])
                    ps = psPO.tile([128, H], F32, tag="po", name="ps_in")
                    nc.tensor.matmul(ps[:], xt[:], w_in[:], start=True, stop=True)
                    if bt is not None:
                        nc.vector.tensor_add(ps[:], ps[:], bt[:])
                    nc.scalar.activation(h[:, t, nt, :], ps[:], AF.Relu)

            def transpose_tile(src2, nt_label):
                """h tile [128, 256] f32 -> hT [128, 2, 128] bf16 (feature-major)."""
                hTt = ws.tile([128, 2, 128], BF16, tag="hTt")
                for ft in range(2):
                    tp = psSC.tile([128, 128], F32, tag="sc")
                    nc.tensor.transpose(tp[:], src2[:, ft * 128:(ft + 1) * 128], ident[:])
                    if (nt_label + ft) % 2:
                        nc.vector.tensor_copy(hTt[:, ft, :], tp[:])
                    else:
                        nc.scalar.copy(hTt[:, ft, :], tp[:])
                return hTt

            for l in range(L):
                # ---- projections ----
                # Pass 1: transposes + kv projections per src type, each
                # followed immediately by its AllGather so both collectives
                # are in flight before any q' work. hT for types 0/1 is kept
                # for pass 2; type 2 transposes inline.
                with nc.named_scope(f"l{l}_proj"):
                    hTbig = pp.tile([128, 2, 2, NT, 128], BF16, tag="hTbig",
                                    name="hTbig")
                    for t in (0, 1):
                        if l == 0:
                            with nc.named_scope(f"inproj{t}"):
                                inproj_type(t)
                        wkt = load_w(wk_d[l, t]); wvt = load_w(wv_d[l, t])
                        bkt = load_bias(bias_d["bk_b"][l, t]) if ub["bkv"] else None
                        bvt = load_bias(bias_d["bv_b"][l, t]) if ub["bkv"] else None
                        for half in range(2):
                            kvstg = stg.tile([128, NT // 2, KV_W], BF16, tag="kvstg",
                                             name="kvstg")
                            for nti in range(NT // 2):
                                nt = half * (NT // 2) + nti
                                for ft in range(2):
                                    tp = psSC.tile([128, 128], F32, tag="sc",
                                                   name="tp")
                                    nc.tensor.transpose(
                                        tp[:], h[:, t, nt, ft * 128:(ft + 1) * 128],
                                        ident[:])
                                    if (nt + ft) % 2:
                                        nc.vector.tensor_copy(hTbig[:, t, ft, nt, :], tp[:])
                                    else:
                                        nc.scalar.copy(hTbig[:, t, ft, nt, :], tp[:])
                                for (wt, bt, col) in ((wkt, bkt, 0), (wvt, bvt, H)):
                                    ps = psPO.tile([128, H], F32, tag="po", name="ps_kv")
                                    for kt in range(2):
                                        nc.tensor.matmul(ps[:], hTbig[:, t, kt, nt, :],
                                                         wt[:, kt, :],
                                                         start=(kt == 0), stop=(kt == 1))
                                    dst_ = kvstg[:, nti, col:col + H]
                                    if bt is not None:
                                        nc.vector.tensor_add(dst_, ps[:], bt[:])
                                    else:
                                        nc.vector.tensor_copy(dst_, ps[:])
                            r0_ = half * (NT // 2) * 128
                            r1_ = r0_ + (NT // 2) * 128
                            nc.sync.dma_start(
                                kv_loc[l][t][r0_:r1_, :].rearrange(
                                    "(nt kp) m -> kp nt m", kp=128),
                                kvstg[:])
                        with nc.named_scope(f"l{l}_ag{t}"):
                            nc.gpsimd.collective_compute(
                                "AllGather", OP.bypass,
                                replica_groups=[list(range(NC))],
                                ins=[kv_loc[l][t][:].opt()],
                                outs=[kv_full[l][t][:].opt()],
                            )
                    # Pass 2: q' projections per relation (edge order), from
                    # the retained hT (types 0/1) or inline transposes (t2).
                    if l == 0:
                        with nc.named_scope("inproj2"):
                            inproj_type(2)
                    for r in EDGE_ORDER:
                        dt_ = DST_T[r]
                        wqr = load_w(wq_d[l, r])
                        bqr = load_bias(bias_d["bq_b"][l, r]) if ub["bq"] else None
                        for half in range(2):
                            qstg = stg.tile([128, NT // 2, H], BF16, tag="qstg",
                                            name=f"qstg{r}")
                            for nti in range(NT // 2):
                                nt = half * (NT // 2) + nti
                                if dt_ < 2:
                                    hTt2 = hTbig[:, dt_, :, nt, :]
                                else:
                                    hTt = transpose_tile(h[:, dt_, nt, :], nt)
                                    hTt2 = hTt[:]
                                ps = psPO.tile([128, H], F32, tag="po", name="ps_q")
                                for kt in range(2):
                                    nc.tensor.matmul(ps[:], hTt2[:, kt, :],
                                                     wqr[:, kt, :],
                                                     start=(kt == 0), stop=(kt == 1))
                                if bqr is not None:
                                    nc.vector.tensor_add(qstg[:, nti, :], ps[:], bqr[:])
                                else:
                                    nc.vector.tensor_copy(qstg[:, nti, :], ps[:])
                            r0_ = half * (NT // 2) * 128
                            r1_ = r0_ + (NT // 2) * 128
                            nc.sync.dma_start(
                                q_dram[l][r, r0_:r1_, :].rearrange(
                                    "(nt kp) m -> kp nt m", kp=128),
                                qstg[:])

                # ---- edge phase; r order: 0 (t1 agg), 1 (t0), 3 (t2), 2 (t1) ----
                s1 = sp.tile([128, NT], F32, tag="s1")
                s2 = sp.tile([128, NT], F32, tag="s2")

                def finish_type(t, l):
                    mu = sp.tile([128, NT], F32, tag="mu")
                    inv = sp.tile([128, NT], F32, tag="inv")
                    nmi = sp.tile([128, NT], F32, tag="nmi")
                    nc.vector.tensor_scalar_mul(mu[:], s1[:], 1.0 / H)
                    nc.vector.tensor_scalar_mul(inv[:], s2[:], 1.0 / H)  # mean sq
                    musq = sp.tile([128, NT], F32, tag="musq")
                    nc.vector.tensor_mul(musq[:], mu[:], mu[:])
                    nc.vector.scalar_tensor_tensor(
                        inv[:], inv[:], float(eps_eff[l][t]), musq[:],
                        OP.add, OP.subtract)              # var + eps
                    nc.scalar.activation(inv[:], inv[:], AF.Sqrt)
                    nc.vector.reciprocal(inv[:], inv[:])
                    nc.vector.scalar_tensor_tensor(
                        nmi[:], mu[:], -1.0, inv[:], OP.mult, OP.mult)
                    if ub["lng"] or ub["lnb"]:
                        lng_t = load_bias(bias_d["lng_b"][l, t])
                        lnb_t = load_bias(bias_d["lnb_b"][l, t])
                        for w in range(NT):
                            nc.scalar.activation(
                                h[:, t, w, :], h[:, t, w, :], AF.Identity,
                                bias=nmi[:, w:w + 1], scale=inv[:, w:w + 1])
                            nc.vector.tensor_mul(h[:, t, w, :], h[:, t, w, :], lng_t[:])
                            nc.vector.tensor_add(h[:, t, w, :], h[:, t, w, :], lnb_t[:])
                            nc.scalar.activation(h[:, t, w, :], h[:, t, w, :], AF.Relu)
                    else:
                        for w in range(NT):
                            nc.scalar.activation(
                                h[:, t, w, :], h[:, t, w, :], AF.Relu,
                                bias=nmi[:, w:w + 1], scale=inv[:, w:w + 1])

                for r in EDGE_ORDER:
                    _sid, _ = nc.enter_named_scope(f"l{l}_r{r}", False)
                    dt = DST_T[r]
                    st = SRC_T[r]
                    wa_t = ba_t = None
                    if r != 0:
                        wa_t = load_w(wa_d[l, dt])
                        ba_t = load_bias(bias_d["ba_b"][l, dt]) if ub["ba"] else None
                    mblk_t = ws.tile([128, 2, 128], BF16, tag="mblk")
                    nc.sync.dma_start(mblk_t[:], m_blk_d[l, r].rearrange("kt p m -> p kt m"))
                    kvi = ip.tile([128, NIDX_R16(KCH)], I16, tag="kvi")
                    qii = ip.tile([128, NIDX_R16(KCH)], I16, tag="qii")
                    nc.sync.dma_start(kvi[:], kv_idx_d[r])
                    nc.sync.dma_start(qii[:], qi_idx_d[r])
                    for gidx in range(NGRP):
                        ni = GC * 128
                        kvg = ep.tile([128, GC, KV_W], BF16, tag="kvg")
                        qig = ep.tile([128, GC, H], BF16, tag="qig")
                        nc.gpsimd.dma_gather(
                            kvg[:], kv_full[l][st][:],
                            kvi[:, gidx * (ni // 16):(gidx + 1) * (ni // 16)],
                            ni, ni, KV_W)
                        nc.gpsimd.dma_gather(
                            qig[:], q_dram[l][r],
                            qii[:, gidx * (ni // 16):(gidx + 1) * (ni // 16)],
                            ni, ni, H)
                        ohg = ep.tile([128, GC, 128], BF16, tag="ohg")
                        nc.sync.dma_start(ohg[:], oh_d[r, :, gidx * GC:(gidx + 1) * GC, :])
                        msg = ep.tile([128, GC, H + HEADS], BF16, tag="msg")
                        if DBG and l == 0 and r == 0 and gidx == 0:
                            nc.sync.dma_start(dbg_kvg, kvg[:])
                            nc.sync.dma_start(dbg_qig, qig[:])
                        lg = sp.tile([128, GC, HEADS], F32, tag="lg")
                        # q*k product staged in msg[:, :, 0:H]; overwritten by
                        # the weighted-v below after the reduce consumes it
                        nc.vector.tensor_mul(msg[:, :, 0:H], qig[:], kvg[:, :, 0:H])
                        nc.vector.tensor_reduce(
                            lg[:], msg[:, :, 0:H].rearrange("p g (hh dd) -> p g hh dd", dd=D),
                            mybir.AxisListType.X, OP.add)
                        nc.scalar.activation(msg[:, :, H:H + HEADS], lg[:], AF.Exp)
                        nc.vector.tensor_mul(
                            msg[:, :, 0:H].rearrange("p g (hh dd) -> p g hh dd", dd=D),
                            kvg[:, :, H:2 * H].rearrange("p g (hh dd) -> p g hh dd", dd=D),
                            bc32(msg[:, :, H:H + HEADS]))
                        if DBG and l == 0 and r == 0 and gidx == 0:
                            nc.sync.dma_start(dbg_msg, msg[:])
                        for wi in range(GWIN):
                            w = gidx * GWIN + wi
                            # scatter: node-major [dst, 256 agg | 8 denom]
                            pw = psSC.tile([128, 264], F32, tag="sc")
                            for kc in range(KCH):
                                ch = wi * KCH + kc
                                nc.tensor.matmul(pw[:], ohg[:, ch, :], msg[:, ch, :],
                                                 start=(kc == 0), stop=(kc == KCH - 1))
                            # rec = 1/denom  [128 dst, 8] bf16
                            # +1e-30: degree-0 dst nodes have sum 0; keep 0*recip = 0
                            recf = sp.tile([128, HEADS], F32, tag="recf")
                            nc.vector.tensor_scalar_add(recf[:], pw[:, H:H + HEADS], 1e-30)
                            rec = sp.tile([128, HEADS], BF16, tag="rec")
                            with nc.allow_low_precision(reason="softmax recip to bf16"):
                                nc.vector.reciprocal(rec[:], recf[:])
                            # normalized node-major agg, bf16
                            an = sp.tile([128, H], BF16, tag="an")
                            nc.vector.tensor_mul(
                                an[:].rearrange("p (hh dd) -> p hh dd", dd=D),
                                pw[:, 0:H].rearrange("p (hh dd) -> p hh dd", dd=D),
                                bc32(rec[:]))
                            # transpose to feature-major for m_rel / Wa
                            anP = psAG.tile([128, 2, 128], BF16, tag="ag")
                            for ft in range(2):
                                nc.tensor.transpose(
                                    anP[:, ft, :], an[:, ft * 128:(ft + 1) * 128],
                                    identb[:])
                            anT = sp.tile([128, 2, 128], BF16, tag="anT")
                            nc.scalar.copy(anT[:], anP[:])
                            # m_rel block-diag transform (feature-major)
                            aggM = psAG.tile([128, 2, 128], F32, tag="ag")
                            for kt in range(2):
                                nc.tensor.matmul(aggM[:, kt, :], mblk_t[:, kt, :],
                                                 anT[:, kt, :], start=True, stop=True)
                            if r == 0:
                                nc.vector.tensor_copy(agg1T[:, :, w, :], aggM[:])
                                if DBG and l == 0 and w == 0:
                                    nc.sync.dma_start(dbg_an, an[:])
                                continue
                            # gelu (exact) in feature-major
                            geluT = sp.tile([128, 2, 128], BF16, tag="geluT")
                            if r == 2:
                                gin = sp.tile([128, 2, 128], F32, tag="gin")
                                nc.vector.tensor_add(gin[:], aggM[:], agg1T[:, :, w, :])
                                nc.scalar.activation(geluT[:], gin[:], AF.Gelu)
                            else:
                                nc.scalar.activation(geluT[:], aggM[:], AF.Gelu)
                            # Wa: node-major output from feature-major gelu
                            po = psPO.tile([128, H], F32, tag="po")
                            for kt in range(2):
                                nc.tensor.matmul(po[:], geluT[:, kt, :], wa_t[:, kt, :],
                                                 start=(kt == 0), stop=(kt == 1))
                            if ba_t is not None:
                                nc.vector.tensor_add(po[:], po[:], ba_t[:])
                            # h_pre = o + h (in place), s1 = row sums
                            nc.vector.scalar_tensor_tensor(
                                h[:, dt, w, :], po[:], 1.0, h[:, dt, w, :],
                                OP.mult, OP.add, accum_out=s1[:, w:w + 1])
                            sqs = sp.tile([128, H], F32, tag="sqs")
                            nc.scalar.activation(sqs[:], h[:, dt, w, :], AF.Square,
                                                 accum_out=s2[:, w:w + 1])
                    if DBG and l == 0 and r == 0:
                        nc.sync.dma_start(dbg_agg, agg1T[:])
                    if r != 0:
                        finish_type(dt, l)
                    nc.leave_named_scope(f"l{l}_r{r}", _sid, False)

                if DBG and l == 0:
                    nc.sync.dma_start(dbg_h, h[:])

            # ---- output projection ----
            with nc.named_scope("outproj"):
                wo = load_w(wout_d)
                bo = load_bias(bias_d["bout_b"]) if ub["bout"] else None
                for t in range(T):
                    for nt in range(NT):
                        hTt = transpose_tile(h[:, t, nt, :], nt)
                        ps = psPO.tile([128, OUT], F32, tag="po")
                        for kt in range(2):
                            nc.tensor.matmul(ps[:], hTt[:, kt, :], wo[:, kt, :OUT],
                                             start=(kt == 0), stop=(kt == 1))
                        st_ = stg.tile([128, OUT], F32, tag="yout")
                        if bo is not None:
                            nc.vector.tensor_add(st_[:], ps[:], bo[:, :OUT])
                        else:
                            nc.scalar.copy(st_[:], ps[:])
                        nc.sync.dma_start(y_d[t, nt * 128:(nt + 1) * 128, :], st_[:])
    nc.compile()
    return nc


def kernel(**inputs):
    shared, per_core, meta = _preprocess(inputs)
    shapes = {k: list(v.shape) for k, v in {**shared, **per_core[0]}.items()}
    nc = bacc.Bacc("TRN2", target_bir_lowering=False, debug=False, num_devices=NC)
    nc = _build(nc, meta, shapes)
    in_maps = [{**shared, **per_core[c]} for c in range(NC)]
    res = run_bass_kernel_spmd(nc, in_maps, core_ids=list(range(NC)))
    y = np.concatenate([res.results[c]["y"][:, :NL, :] for c in range(NC)], axis=1)
    return y.astype(np.float32)


if __name__ == "__main__":
    import reference
    inputs = {k: np.asarray(v) for k, v in reference.setup_inputs().items()}
    out = kernel(**inputs)
    exp = np.asarray(reference.reference(**inputs))
    err = np.abs(out - exp).max() / np.abs(exp).max()
    print("Relative error:", err)
